# revision 130
# baseline (speedup 1.0000x reference)
"""Trainium2 Bass kernel for MimiAttention (GQA + RoPE + causal softmax).

Problem: B=2, S=2048, H=1024, NH=16 q-heads, NKV=4 kv-heads, HD=64.
Sharding: 8 cores = 2 (batch) x 4 (kv-group).  Each core computes one batch's
attention for one GQA group (4 q-heads sharing 1 kv head) and the partial
o-projection for those heads; the host sums the 4 partials per batch.

Design (all matmuls bf16 with fp32 psum; ~127us cost-model time per core):
  * Packed q-projections: q heads are projected 2-at-a-time with UNduplicated
    weights ([q_h; q_h+1] on 128 psum rows), then each head's RoPE layout
    [q; q2] (q2 = sign-permuted rows) is recovered with a single 128x128
    permute matmul and multiplied by cs = [cos; sin].  This cuts q-projection
    PE cycles ~1.6x vs. duplicated-weight columns.  khat = [k_rot; k_rot]
    via the J-fold matmul as before.
  * PE p-state warmup: ~44 matmuls against a memset tile burn the tensor
    engine's slow-clock ramp while the first input DMAs are in flight.
  * Head 0 walks key rows DESCENDING while xt streams column-blocks in
    reverse (block 3 first, kc-half granularity): row jt only needs xt
    columns >= 128*jt, so each arriving block unlocks the next 4 rows plus
    their k/v/pair-projection chunks and the PE never waits long for data.
    All 16 attnV slices then close at row 0 and are normalized in one burst
    (reciprocals batched first, then multiplies, all on DVE, descending
    slice order to pipeline with head 1's bank-clearing writes).  Heads 1-3
    ascend, with per-row slice normalization as usual.
  * Scores computed transposed (scoresT[j, i]) one key-tile row at a time,
    streamed through two ping-pong [128, 1024] PSUM feed regions; ONE exp
    activation per <=1024-col segment minimizes the ACT engine's fixed
    per-instruction cost; rows wider than 1024 split into BALANCED halves
    so consecutive rows chase similar-sized exps through the ping-pong.  Feed slots are also "borrowed" (with the same
    parity rotation) as scratch psum for pair/permute/v work so producer
    chains don't serialize on the single work bank.
  * Software pipeline: scores+exp for row r are issued before attnV of row
    r-1; the causal diagonal is masked in place on the exp output (DVE
    2x-mode mult, Pool for head 3); 9-deep et tile rotation.
  * attnV accumulates out[i, v|den] slices in 3 persistent PSUM banks
    (65-wide slices; column 64 = softmax denominator via a ones-column in
    v).
  * o-projection streams during head 3; output splits: seq 0:1536 as
    [H, 1536] bf16 with 4-hc-batched [512,512] DMAs, seq 1536:1920 (query
    tiles 12-14, ready one attnV row before the end) and the final 128-wide
    tile-15 sliver as flat bf16 tensors so the kernel tail is two small
    transfers.  The host reassembles/stitches the three pieces.
"""

import numpy as np
import ml_dtypes

B, S, H = 2, 2048, 1024
NH, NKV, HD = 16, 4, 64
G = NH // NKV            # 4 q-heads per kv head
THETA = 10000.0
N_CORES = 8

BF16 = ml_dtypes.bfloat16

NSB = S // 512           # 4 chunks of 512
NST = S // 128           # 16 tiles of 128
KC = H // 128            # 8 contraction chunks
SCALE = float(1.0 / np.sqrt(HD))
N_WARM = 40              # PE p-state warmup matmuls (128 cols each)


LABEL_MAP = {}


def _build_nc():
    import concourse.mybir as mybir
    import concourse.tile as tile
    from concourse.tile import add_dep_helper
    from concourse import bacc

    f32 = mybir.dt.float32
    bf16 = mybir.dt.bfloat16

    nc = bacc.Bacc("TRN2", target_bir_lowering=False)

    # instruction-name -> source-op label, for the timeline analyzer
    LABEL_MAP.clear()
    _cur = ["init"]

    def L(s):
        _cur[0] = s

    _orig_gnin = nc.get_next_instruction_name

    def _gnin():
        nm = _orig_gnin()
        LABEL_MAP[nm] = _cur[0]
        return nm

    nc.get_next_instruction_name = _gnin

    xTd = nc.dram_tensor("xT", [H, S], bf16, kind="ExternalInput")
    wk8d = nc.dram_tensor("wkT8", [128, KC * 128], bf16, kind="ExternalInput")
    # q weights packed 2 heads per 128 columns (no RoPE duplication); the
    # per-head [q; q2] layout is recovered with one sign-permute matmul.
    wp0d = nc.dram_tensor("wp0T8", [128, KC * 128], bf16,
                          kind="ExternalInput")
    wp1d = nc.dram_tensor("wp1T8", [128, KC * 128], bf16,
                          kind="ExternalInput")
    pm0d = nc.dram_tensor("perm0", [128, 128], bf16, kind="ExternalInput")
    pm1d = nc.dram_tensor("perm1", [128, 128], bf16, kind="ExternalInput")
    wvd = nc.dram_tensor("wvT", [H, HD], bf16, kind="ExternalInput")
    csd = nc.dram_tensor("cs", [128, S], bf16, kind="ExternalInput")
    wod = nc.dram_tensor("woT", [G * HD, H], bf16, kind="ExternalInput")
    trid = nc.dram_tensor("trimask", [128, 128], bf16, kind="ExternalInput")
    djd = nc.dram_tensor("dupJ", [128, 128], bf16, kind="ExternalInput")
    idd = nc.dram_tensor("ident", [128, 128], bf16, kind="ExternalInput")
    # outputs: main seq cols 0:1536 as [H, 1536] bf16; seq 1536:2048 flat
    # bf16, per query tile ([feat%128, tile, half, hcg, s]) so each of the
    # last 4 query tiles streams out as soon as its attnV row completes.
    # The host reassembles.
    oTd = nc.dram_tensor("oT", [H, 3 * 512], bf16, kind="ExternalOutput")
    oT3d = nc.dram_tensor("oT3", [128, 2 * 4 * 384], bf16,
                          kind="ExternalOutput")
    oT2d = nc.dram_tensor("oT2", [128, 8 * 128], bf16, kind="ExternalOutput")

    with tile.TileContext(nc) as tc:
        import contextlib
        ctx = contextlib.ExitStack()
        with ctx:
            consts = ctx.enter_context(tc.tile_pool(name="consts", bufs=1))
            acts = ctx.enter_context(tc.tile_pool(name="acts", bufs=1))
            ep = ctx.enter_context(tc.tile_pool(name="exps", bufs=4))
            rcp = ctx.enter_context(tc.tile_pool(name="rcp", bufs=20))
            otp = ctx.enter_context(tc.tile_pool(name="ot", bufs=8))
            pav = ctx.enter_context(
                tc.tile_pool(name="ps_av", bufs=1, space="PSUM"))
            pfa = ctx.enter_context(
                tc.tile_pool(name="ps_fa", bufs=1, space="PSUM"))
            pfb = ctx.enter_context(
                tc.tile_pool(name="ps_fb", bufs=1, space="PSUM"))
            pw = ctx.enter_context(
                tc.tile_pool(name="ps_w", bufs=1, space="PSUM"))

            # ---- input DMAs, ordered by first use: k weights + first xt
            # column block feed the k/q0 projections; the remaining xt lands
            # column-major so qhat chunks stream in order.  wk/wq0 use
            # host-preswizzled contiguous [128, KC*128] layouts so their DMA
            # descriptors are 2KB (no sub-512B penalty), and the first xt
            # column block is split into kc-pair chunks so the k projection
            # can start after ~1/4 of it has landed.
            xt_sb = consts.tile([128, KC, S], bf16, tag="xt")
            wk_sb = consts.tile([128, KC * 128], bf16, tag="wk")
            wp_sb = [consts.tile([128, KC * 128], bf16, tag=f"wp{p}",
                                 name=f"wp{p}") for p in range(2)]
            pm_sb = [consts.tile([128, 128], bf16, tag=f"pm{p}",
                                 name=f"pm{p}") for p in range(2)]
            cs_sb = consts.tile([128, S], bf16, tag="cs")
            tri_sb = consts.tile([128, 128], bf16, tag="tri")
            dj_sb = consts.tile([128, 128], bf16, tag="dj")
            id_sb = consts.tile([128, 128], bf16, tag="id")
            wv_sb = consts.tile([128, KC, HD], bf16, tag="wv")
            wo_sb = consts.tile([128, 2, H], bf16, tag="wo")
            warm_sb = consts.tile([128, 128], bf16, tag="warm")

            # PE warmup: burn the p-state ramp against a memset tile while
            # the first input DMAs are in flight.
            nc.gpsimd.memset(warm_sb, 0.0)
            warm_ps = pw.tile([128, 128], f32, tag="w", name="warmps")
            for _ in range(N_WARM):
                nc.tensor.matmul(warm_ps, warm_sb, warm_sb,
                                 start=True, stop=True, skip_group_check=True)

            def xt_col(n):
                # two half-blocks so dependent projections can chase
                for half in range(2):
                    c = n * 512
                    r0 = half * 512
                    nc.sync.dma_start(
                        xt_sb[:, 4 * half:4 * half + 4, c:c + 512],
                        xTd[r0:r0 + 512, c:c + 512].rearrange(
                            "(kc p) m -> p kc m", p=128))

            # head 0 consumes key rows DESCENDING, so xt streams in reverse
            # column order: each arriving 512-col block unlocks the next 4
            # rows plus their k/v/q projection chunks.
            nc.sync.dma_start(wp_sb[0], wp0d[:, :])
            nc.sync.dma_start(wk_sb, wk8d[:, :])
            for kk in range(4):
                r0 = kk * 256
                nc.sync.dma_start(
                    xt_sb[:, 2 * kk:2 * kk + 2, 1536:2048],
                    xTd[r0:r0 + 256, 1536:2048].rearrange(
                        "(kc p) m -> p kc m", p=128))
                if kk == 0:
                    nc.sync.dma_start(pm_sb[0], pm0d[:, :])
                if kk == 1:
                    nc.sync.dma_start(
                        cs_sb[:, 1024:2048], csd[:, 1024:2048])
            nc.sync.dma_start(dj_sb, djd[:, :])
            nc.sync.dma_start(tri_sb, trid[:, :])
            nc.sync.dma_start(wv_sb, wvd.rearrange("(kc p) m -> p kc m", p=128))
            xt_col(2)
            nc.sync.dma_start(cs_sb[:, 0:1024], csd[:, 0:1024])
            xt_col(1)
            xt_col(0)
            nc.sync.dma_start(pm_sb[1], pm1d[:, :])
            nc.sync.dma_start(wp_sb[1], wp1d[:, :])
            nc.sync.dma_start(wo_sb, wod.rearrange("(kc p) m -> p kc m", p=128))
            nc.sync.dma_start(id_sb, idd[:, :])

            qhat = [acts.tile([128, S], bf16, tag=f"qh{m}", name=f"qhat{m}")
                    for m in range(G)]
            qpk = acts.tile([128, 2, S], bf16, tag="qpk")
            khat = acts.tile([128, S], bf16, tag="khat")
            ktmp = acts.tile([128, S], bf16, tag="ktmp")
            v_sb = acts.tile([128, NST, HD + 1], bf16, tag="vsb")
            attn_n = acts.tile([128, NST, G * HD], bf16, tag="attn")
            aT = acts.tile([128, 2, S], bf16, tag="aT")

            avb = [pav.tile([128, w], f32, tag=f"av{b}", name=f"avb{b}")
                   for b, w in ((0, 455), (1, 455), (2, 130))]

            def av_slice(it):
                b, o = it // 7, (it % 7) * 65
                return avb[b][:, o:o + 65]

            seg_counter = [0]

            def feed_tile(idx, ln):
                # ping-pong exp-feed regions, allocated per segment so the
                # pool slot rotation provides the WAR chain
                if idx % 2 == 0:
                    return pfa.tile([128, ln], f32, tag="fA", name="feed",
                                    padded_shape=[128, 1024])
                return pfb.tile([128, ln], f32, tag="fB", name="feed",
                                padded_shape=[128, 1024])

            def proj_psum(lhs_sb, n, ps):
                col = n * 512
                for kc in range(KC):
                    nc.tensor.matmul(
                        ps, lhs_sb[:, kc * 128:(kc + 1) * 128],
                        xt_sb[:, kc, col:col + 512],
                        start=(kc == 0), stop=(kc == KC - 1))

            def borrow_feed():
                # take the next feed slot out of the scores ping-pong; its
                # pool-rotation WAR lines up with the borrower's own deps
                ps = feed_tile(seg_counter[0], 512)
                seg_counter[0] += 1
                return ps

            def pair_proj(pr, n, ps=None):
                # raw packed projection of q-heads (2pr, 2pr+1); the psum is
                # copied to sbuf so the per-head permutes can stream from it
                L(f"pair_proj{pr}_{n}")
                if ps is None:
                    ps = pw.tile([128, 512], f32, tag="w", name="pspp")
                proj_psum(wp_sb[pr], n, ps)
                col = n * 512
                nc.vector.tensor_copy(qpk[:, pr, col:col + 512], ps)

            def q_permute(h, n, ps=None):
                # [q_h; sign-permuted q_h] via one 128x128 matmul, then the
                # RoPE cos/sin multiply
                L(f"q_permute{h}_{n}")
                col = n * 512
                if ps is None:
                    ps = pw.tile([128, 512], f32, tag="w", name="psqp")
                nc.tensor.matmul(ps, pm_sb[h % 2],
                                 qpk[:, h // 2, col:col + 512],
                                 start=True, stop=True)
                nc.vector.tensor_mul(
                    qhat[h][:, col:col + 512], ps, cs_sb[:, col:col + 512])

            def k_proj(n, ps=None):
                L(f"k_proj{n}")
                if ps is None:
                    ps = pw.tile([128, 512], f32, tag="w", name="psk")
                proj_psum(wk_sb, n, ps)
                col = n * 512
                nc.vector.tensor_mul(
                    ktmp[:, col:col + 512], ps, cs_sb[:, col:col + 512])

            def k_fold(n, psf=None, eng=0):
                L(f"k_fold{n}")
                col = n * 512
                if psf is None:
                    psf = pw.tile([128, 512], f32, tag="w", name="psf")
                nc.tensor.matmul(psf, dj_sb, ktmp[:, col:col + 512],
                                 start=True, stop=True)
                if eng == 0:
                    nc.vector.tensor_copy(khat[:, col:col + 512], psf)
                else:
                    nc.scalar.copy(khat[:, col:col + 512], psf)

            def k_chunk(n, ps=None, psf=None):
                k_proj(n, ps)
                k_fold(n, psf)

            def v_tiles(st0, nt, ps=None):
                L(f"v_tiles{st0}")
                # project nt seq-tiles of v through one work-psum residency
                if ps is None:
                    psv = pw.tile([128, nt, HD], f32, tag="w", name="psv",
                                  padded_shape=[128, 4, HD])
                else:
                    psv = ps.rearrange("p (t d) -> p t d", d=HD)[:, 0:nt, :]
                for t in range(nt):
                    st = st0 + t
                    for kc in range(KC):
                        nc.tensor.matmul(
                            psv[:, t, :],
                            xt_sb[:, kc, st * 128:(st + 1) * 128],
                            wv_sb[:, kc, :],
                            start=(t == 0 and kc == 0), stop=(kc == KC - 1),
                            skip_group_check=True)
                nc.vector.tensor_copy(
                    v_sb[:, st0:st0 + nt, 0:HD], psv)

            def transpose_tiles(hp, its):
                L(f"transp{hp}_{its[0]}")
                # slice transposes through one work-psum residency
                psx = pw.tile([128, len(its), 128], bf16, tag="w", name="pst",
                              padded_shape=[128, 4, 128])
                for t, it in enumerate(its):
                    nc.tensor.matmul(
                        psx[:, t, :], attn_n[:, it, hp * 128:(hp + 1) * 128],
                        id_sb, is_transpose=True,
                        start=(t == 0), stop=True, skip_group_check=True)
                c0 = its[0] * 128
                nc.vector.tensor_copy(
                    aT[:, hp, c0:c0 + len(its) * 128], psx)

            def transpose_group(hp, g4):
                transpose_tiles(hp, list(range(g4 * 4, g4 * 4 + 4)))

            # ---- prologue: chase the reverse-streamed xt column block 3.
            # k chunk 3 and pair0 chunk 3 interleave per-kc so both finish
            # right after the last xt3 sub-block lands.
            nc.gpsimd.memset(v_sb[:, :, HD:HD + 1], 1.0)
            L("k3+p03")
            psk3 = feed_tile(0, 512)
            psp3 = feed_tile(1, 512)
            for kc in range(KC):
                nc.tensor.matmul(
                    psk3, wk_sb[:, kc * 128:(kc + 1) * 128],
                    xt_sb[:, kc, 1536:2048],
                    start=(kc == 0), stop=(kc == KC - 1))
                nc.tensor.matmul(
                    psp3, wp_sb[0][:, kc * 128:(kc + 1) * 128],
                    xt_sb[:, kc, 1536:2048],
                    start=(kc == 0), stop=(kc == KC - 1))
            nc.vector.tensor_mul(ktmp[:, 1536:2048], psk3, cs_sb[:, 1536:2048])
            nc.scalar.copy(qpk[:, 0, 1536:2048], psp3)
            seg_counter[0] = 2
            k_fold(3, psf=pw.tile([128, 512], f32, tag="w", name="psf3"))
            q_permute(0, 3, ps=borrow_feed())

            def scores_row(h, jt, et, segs=None, cbs=None):
                L(f"scores{h}_{jt}")
                lo = jt * 128
                cols = S - lo
                lhsT = khat[:, lo:lo + 128]
                if segs is None:
                    if cols > 1024:
                        h1len = ((cols // 2 + 127) // 128) * 128
                        segs = [(lo, h1len), (lo + h1len, cols - h1len)]
                    else:
                        segs = [(lo, cols)]
                for si, (off, ln) in enumerate(segs):
                    region = feed_tile(seg_counter[0], ln)
                    seg_counter[0] += 1
                    done = 0
                    while done < ln:
                        cl = min(512, ln - done)
                        nc.tensor.matmul(
                            region[:, done:done + cl], lhsT,
                            qhat[h][:, off + done:off + done + cl],
                            start=True, stop=True)
                        done += cl
                    with tc.high_priority(offset=64):
                        nc.scalar.activation(
                            et[:, off:off + ln], region[:, 0:ln],
                            mybir.ActivationFunctionType.Exp, scale=SCALE)
                    if cbs is not None and si in cbs:
                        cbs[si]()
                # causal mask on diag tile: Pool, hidden by the pipeline
                L(f"mask{h}_{jt}")
                if h == 3:
                    nc.gpsimd.tensor_mul(et[:, lo:lo + 128],
                                         et[:, lo:lo + 128], tri_sb)
                else:
                    nc.vector.tensor_mul(et[:, lo:lo + 128],
                                         et[:, lo:lo + 128], tri_sb)

            attnv_state = {}   # h -> bank_first dict

            def attnv_row(h, jt, et, desc=False):
                # In ascending key order slice jt is complete after this row
                # (stop + normalize); in descending order every slice stays
                # open until row 0 and normalization happens afterwards.
                L(f"attnv{h}_{jt}")
                bank_first = attnv_state.setdefault(h, {})
                b1_hi = min(jt + 7, NST - 1)
                order = list(range(b1_hi, jt - 1, -1)) + \
                    list(range(NST - 1, b1_hi, -1))
                for it in order:
                    b = it // 7
                    first = b not in bank_first
                    mm = nc.tensor.matmul(
                        av_slice(it), et[:, it * 128:(it + 1) * 128],
                        v_sb[:, jt, :],
                        start=first,
                        stop=(jt == 0 if desc else it == jt),
                        skip_group_check=True)
                    if first:
                        bank_first[b] = mm
                    elif jt == 0:
                        add_dep_helper(mm.ins, bank_first[b].ins,
                                       sync=False,
                                       reason="bank clear first")
                if not desc:
                    normalize_slice(h, jt)

            def normalize_slice(h, jt, eng=0, rc=None):
                L(f"norm{h}_{jt}")
                pso = av_slice(jt)
                if rc is None:
                    rc = rcp.tile([128, 1], f32, tag="rc", name="rc")
                    nc.vector.reciprocal(rc, pso[:, HD:HD + 1])
                if eng == 0:
                    nc.vector.tensor_scalar_mul(
                        attn_n[:, jt, h * HD:(h + 1) * HD], pso[:, 0:HD], rc)
                else:
                    nc.scalar.mul(
                        attn_n[:, jt, h * HD:(h + 1) * HD], pso[:, 0:HD], rc)

            # oproj -------------------------------------------------------
            # Column groups g=0..2 keep the original 4-hc-batched [512,512]
            # output DMAs (few HWDGE entries).  Group 3 (seq 1536:2048) is
            # split: a 384-wide part (query tiles 12-14, ready one attnV row
            # before the end) drained + DMA'd flat, and a final 128-wide
            # sliver (tile 15) that is DMA'd directly from PSUM as f32 so
            # the kernel tail is one small transfer with no drain wait.
            oproj_pending = [(g, hc) for g in range(3) for hc in range(KC)]
            ot_state = {}

            def oproj_chunk(ps, drain_eng):
                g, hc = oproj_pending.pop(0)
                L(f"oproj{g}_{hc}")
                col = g * 512
                for kc2 in range(2):
                    nc.tensor.matmul(
                        ps, wo_sb[:, kc2, hc * 128:(hc + 1) * 128],
                        aT[:, kc2, col:col + 512],
                        start=(kc2 == 0), stop=(kc2 == 1))
                if hc % 4 == 0:
                    ot_state[g] = otp.tile([128, 4, 512], bf16, tag="otb",
                                           name="otb")
                ot = ot_state[g]
                with tc.high_priority(offset=-64):
                    if drain_eng == 0:
                        nc.vector.tensor_copy(ot[:, hc % 4, :], ps)
                    elif drain_eng == 1:
                        nc.scalar.copy(ot[:, hc % 4, :], ps)
                    else:
                        nc.vector.tensor_copy(ot[:, hc % 4, 0:256],
                                              ps[:, 0:256])
                        nc.scalar.copy(ot[:, hc % 4, 256:512],
                                       ps[:, 256:512])
                if hc % 4 == 3:
                    r0 = (hc // 4) * 512
                    nc.sync.dma_start(
                        oTd[r0:r0 + 512, col:col + 512].rearrange(
                            "(c p) m -> p c m", p=128), ot)

            g3_pending = list(range(KC))
            ot3_state = {}

            def g3_chunk(ps, drain_eng):
                hc = g3_pending.pop(0)
                L(f"g3_{hc}")
                for kc2 in range(2):
                    nc.tensor.matmul(
                        ps[:, 0:384], wo_sb[:, kc2, hc * 128:(hc + 1) * 128],
                        aT[:, kc2, 1536:1920],
                        start=(kc2 == 0), stop=(kc2 == 1))
                if hc % 4 == 0:
                    ot3_state[hc // 4] = otp.tile(
                        [128, 4, 384], bf16, tag="ot3", name="ot3")
                ot = ot3_state[hc // 4]
                if drain_eng == 0:
                    nc.vector.tensor_copy(ot[:, hc % 4, :], ps[:, 0:384])
                else:
                    nc.scalar.copy(ot[:, hc % 4, :], ps[:, 0:384])
                if hc % 4 == 3:
                    grp = hc // 4
                    nc.sync.dma_start(
                        oT3d[:, grp * 1536:(grp + 1) * 1536], ot)

            # ---- main pipelined loop ------------------------------------
            # head 0 walks key rows DESCENDING (matched to the reverse xt
            # stream: each xt column block unlocks 4 more rows and their
            # k/v/pair-projection chunks); heads 1-3 ascend as before.
            seq = [(0, jt) for jt in range(NST - 1, -1, -1)] + \
                [(h, jt) for h in range(1, G) for jt in range(NST)]
            prev = None
            for (h, jt) in seq:
                et = ep.tile([128, S], bf16, tag="e", name=f"e{h}_{jt}")
                scores_row(h, jt, et)
                if prev is not None:
                    attnv_row(prev[0], prev[1], prev[2], desc=(prev[0] == 0))
                    if prev[:2] == (0, 0):
                        # head 0 ran descending: all 16 attnV slices close
                        # at row 0; normalize split across DVE/ACT, in
                        # descending slice order to pipeline with head 1's
                        # bank-clearing attnV writes (slice 15 first).
                        # Reciprocals batch first so the ACT muls never wait
                        # on an individual DVE recip.
                        rcs = {}
                        for i in range(NST - 1, -1, -1):
                            rcs[i] = rcp.tile([128, 1], f32, tag="rc",
                                              name="rcb")
                            nc.vector.reciprocal(
                                rcs[i], av_slice(i)[:, HD:HD + 1])
                        for i in range(NST - 1, -1, -1):
                            normalize_slice(0, i, eng=0, rc=rcs[i])
                prev = (h, jt, et)

                # interleaved producer work; head-0 rows chase the reverse
                # xt stream, so the projection bursts sit right after the
                # last row that only needs already-landed data.
                if h == 0:
                    if jt == 15:
                        v_tiles(14, 2, ps=borrow_feed())
                    if jt == 14:
                        v_tiles(12, 2, ps=borrow_feed())
                    if jt == 13:
                        k_proj(2)
                    if jt == 12:
                        k_fold(2)
                        pair_proj(0, 2, ps=borrow_feed())
                        q_permute(0, 2, ps=borrow_feed())
                    if jt == 11:
                        v_tiles(8, 4, ps=borrow_feed())
                    if jt == 9:
                        k_proj(1)
                    if jt == 8:
                        k_fold(1)
                        pair_proj(0, 1, ps=borrow_feed())
                        q_permute(0, 1, ps=borrow_feed())
                    if jt == 7:
                        v_tiles(4, 4)
                    if jt == 5:
                        k_proj(0)
                    if jt == 4:
                        k_fold(0)
                        pair_proj(0, 0, ps=borrow_feed())
                        q_permute(0, 0, ps=borrow_feed())
                    if jt == 3:
                        v_tiles(0, 4)
                    if jt in (3, 2, 1, 0):
                        q_permute(1, 3 - jt)
                if h == 1:
                    if jt in (1, 4, 7, 10):
                        pair_proj(1, (jt - 1) // 3)
                    if jt in (2, 5, 8, 11):
                        q_permute(2, (jt - 2) // 3)
                if h == 2 and jt in (1, 4, 7, 10):
                    q_permute(3, (jt - 1) // 3)
                if h == 2 and jt in (3, 7, 11, 15):
                    transpose_group(0, jt // 4)
                if h == 3:
                    if jt in (5, 9, 13):
                        transpose_group(1, (jt - 5) // 4)
                    if jt >= 5 and oproj_pending and \
                            oproj_pending[0][0] * 4 + 5 <= jt:
                        ps = pw.tile([128, 512], f32, tag="w", name="psow")
                        oproj_chunk(ps, drain_eng=0)
                    if jt >= 8 and oproj_pending and \
                            oproj_pending[0][0] * 4 + 5 <= jt:
                        ps = pav.tile([128, 512], f32, tag="av0",
                                      name="psoa")
                        oproj_chunk(ps, drain_eng=1 if jt >= 12 else 0)
                    if jt in (11, 13, 15) and oproj_pending and \
                            oproj_pending[0][0] * 4 + 5 <= jt:
                        ps = pav.tile([128, 512], f32, tag="av0",
                                      name="psoa2")
                        oproj_chunk(ps, drain_eng=1)
                    if jt in (14, 15) and oproj_pending and \
                            oproj_pending[0][0] * 4 + 5 <= jt:
                        ps = pav.tile([128, 512], f32, tag="av1",
                                      name="psob")
                        oproj_chunk(ps, drain_eng=1)
                    if jt == 15:
                        # rows 12-14 of head 3 are normalized; pair-1
                        # transposes for tiles 12-14 unblock the 384-wide
                        # part of column group 3.
                        transpose_tiles(1, [12, 13, 14])

            # flush: last attnV row, then the 384-wide part of group 3
            # (query tiles 12-14), the tile-15 transpose, and the sliver.
            attnv_row(*prev)
            ti = 0
            slots = ["fA", "fB", "av0", "w", "av1"]
            pools = {"w": pw, "fA": pfa, "fB": pfb, "av0": pav, "av1": pav}
            while oproj_pending:
                tag = slots[ti % len(slots)]
                ps = pools[tag].tile([128, 512], f32, tag=tag, name="psot")
                oproj_chunk(ps, drain_eng=ti % 2)
                ti += 1

            def g3_next(drain_eng):
                tag = slots[ti % len(slots)]
                ps = pools[tag].tile([128, 512], f32, tag=tag, name="psog3")
                g3_chunk(ps, drain_eng)

            # two g3 chunks cover the normalize latency of row 15, then the
            # tile-15 transpose slots in, then the rest.
            g3_next(0)
            g3_next(1)
            ti += 2
            transpose_tiles(1, [15])
            while g3_pending:
                g3_next(ti % 2)
                ti += 1
            del g3_next

            # sliver: 8 feature chunks x 128 seq cols; two [128, 512] f32
            # psum tiles, drained on parallel engines, two small flat DMAs.
            for half in range(2):
                L(f"sliver{half}")
                psl = (pfa if half == 0 else pfb).tile(
                    [128, 512], f32, tag=("fA" if half == 0 else "fB"),
                    name="psliv", padded_shape=[128, 1024])
                for sub in range(4):
                    hc = half * 4 + sub
                    for kc2 in range(2):
                        nc.tensor.matmul(
                            psl[:, sub * 128:(sub + 1) * 128],
                            wo_sb[:, kc2, hc * 128:(hc + 1) * 128],
                            aT[:, kc2, 1920:2048],
                            start=(kc2 == 0), stop=(kc2 == 1))
                ot2 = otp.tile([128, 512], bf16, tag="ot2", name="ot2")
                if half == 0:
                    nc.vector.tensor_copy(ot2, psl)
                else:
                    nc.scalar.copy(ot2, psl)
                nc.sync.dma_start(
                    oT2d[:, half * 512:(half + 1) * 512], ot2)

    nc.finalize()
    return nc


def _host_inputs(hidden_states, position_ids, wq, wk, wv, wo):
    """Build the 8 per-core input maps."""
    def w2_of(w):
        # w: [64, H] rows of one head; returns sign-permuted rows
        w2 = np.empty_like(w)
        w2[:32] = -w[32:64]
        w2[32:] = w[:32]
        return w2

    trimask = np.triu(np.ones((128, 128), np.float32)).astype(BF16)
    dupJ = np.zeros((128, 128), np.float32)
    for p in range(128):
        dupJ[p, p % 64] = 1.0
        dupJ[p, p % 64 + 64] = 1.0
    dupJ = dupJ.astype(BF16)
    ident = np.eye(128, dtype=np.float32).astype(BF16)

    # perm[lh][k, p]: rows of the packed pair projection (head lh occupies
    # input rows 64*lh..64*lh+63) -> [q; sign-permuted q] output rows
    perms = []
    for lh in range(2):
        P = np.zeros((128, 128), np.float32)
        b = 64 * lh
        for p in range(64):
            P[b + p, p] = 1.0
        for i in range(32):
            P[b + 32 + i, 64 + i] = -1.0
            P[b + i, 96 + i] = 1.0
        perms.append(P.astype(BF16))

    def swz(wT):
        # [H, 128] -> [128, KC*128] with 2KB-contiguous DMA descriptors
        return np.ascontiguousarray(
            wT.reshape(KC, 128, 128).transpose(1, 0, 2)
            .reshape(128, KC * 128)).astype(BF16)

    in_maps = []
    for core in range(N_CORES):
        b, kv = core // NKV, core % NKV
        xT = np.ascontiguousarray(hidden_states[b].T).astype(BF16)

        wkh = wk[kv * HD:(kv + 1) * HD]
        wkT8 = swz(np.concatenate([wkh.T, w2_of(wkh).T], axis=1))
        wpT8 = []
        for pr in range(2):
            h0 = kv * G + 2 * pr
            wpT8.append(swz(wq[h0 * HD:(h0 + 2) * HD].T))

        wvT = np.ascontiguousarray(wv[kv * HD:(kv + 1) * HD].T).astype(BF16)
        woT = np.ascontiguousarray(
            wo[:, kv * G * HD:(kv + 1) * G * HD].T).astype(BF16)

        inv = 1.0 / (THETA ** (np.arange(0, HD, 2, dtype=np.float32) / HD))
        freqs = position_ids[b].astype(np.float32)[:, None] * inv[None, :]
        emb = np.concatenate([freqs, freqs], axis=-1)       # [S, 64]
        cs = np.concatenate([np.cos(emb).T, np.sin(emb).T], axis=0)  # [128, S]
        cs = np.ascontiguousarray(cs).astype(BF16)

        in_maps.append({
            "xT": xT, "wvT": wvT, "cs": cs, "woT": woT,
            "trimask": trimask, "dupJ": dupJ, "ident": ident,
            "wkT8": wkT8, "wp0T8": wpT8[0], "wp1T8": wpT8[1],
            "perm0": perms[0], "perm1": perms[1],
        })
    return in_maps


_NC_CACHE = {}


def run_cores(in_maps, trace=False, trace_kwargs=None):
    from concourse.bass_utils import run_bass_kernel_spmd
    if "nc" not in _NC_CACHE:
        _NC_CACHE["nc"] = _build_nc()
    nc = _NC_CACHE["nc"]
    return run_bass_kernel_spmd(
        nc, in_maps, core_ids=list(range(N_CORES)),
        trace=trace, **(trace_kwargs or {}))


def assemble(res):
    """Gather the 3 per-core output pieces into the full [B, S, H] output."""
    out = np.zeros((B, S, H), np.float32)
    for core in range(N_CORES):
        b = core // NKV
        r = res.results[core]
        out[b, 0:1536] += r["oT"].T.astype(np.float32)
        out[b, 1536:1920] += (
            r["oT3"].reshape(128, 2, 4, 384).transpose(3, 1, 2, 0)
            .reshape(384, H).astype(np.float32))
        out[b, 1920:2048] += (
            r["oT2"].reshape(128, 8, 128).transpose(2, 1, 0)
            .reshape(128, H).astype(np.float32))
    return out


def kernel(hidden_states, attention_mask, position_ids, wq, wk, wv, wo):
    hidden_states = np.asarray(hidden_states, dtype=np.float32)
    position_ids = np.asarray(position_ids)
    wq = np.asarray(wq, dtype=np.float32)
    wk = np.asarray(wk, dtype=np.float32)
    wv = np.asarray(wv, dtype=np.float32)
    wo = np.asarray(wo, dtype=np.float32)

    in_maps = _host_inputs(hidden_states, position_ids, wq, wk, wv, wo)
    res = run_cores(in_maps)
    return assemble(res)



# revision 132
# speedup vs baseline: 1.0079x; 1.0079x over previous
"""Trainium2 Bass kernel for MimiAttention (GQA + RoPE + causal softmax).

Problem: B=2, S=2048, H=1024, NH=16 q-heads, NKV=4 kv-heads, HD=64.
Sharding: 8 cores = 2 (batch) x 4 (kv-group).  Each core computes one batch's
attention for one GQA group (4 q-heads sharing 1 kv head) and the partial
o-projection for those heads; the host sums the 4 partials per batch.

Design (all matmuls bf16 with fp32 psum; ~127us cost-model time per core):
  * Packed q-projections: q heads are projected 2-at-a-time with UNduplicated
    weights ([q_h; q_h+1] on 128 psum rows), then each head's RoPE layout
    [q; q2] (q2 = sign-permuted rows) is recovered with a single 128x128
    permute matmul and multiplied by cs = [cos; sin].  This cuts q-projection
    PE cycles ~1.6x vs. duplicated-weight columns.  khat = [k_rot; k_rot]
    via the J-fold matmul as before.
  * PE p-state warmup: ~44 matmuls against a memset tile burn the tensor
    engine's slow-clock ramp while the first input DMAs are in flight.
  * Head 0 walks key rows DESCENDING while xt streams column-blocks in
    reverse (block 3 first, kc-half granularity): row jt only needs xt
    columns >= 128*jt, so each arriving block unlocks the next 4 rows plus
    their k/v/pair-projection chunks and the PE never waits long for data.
    All 16 attnV slices then close at row 0 and are normalized in one burst
    (reciprocals batched first, then multiplies, all on DVE, descending
    slice order to pipeline with head 1's bank-clearing writes).  Heads 1-3
    ascend, with per-row slice normalization as usual.
  * Scores computed transposed (scoresT[j, i]) one key-tile row at a time,
    streamed through two ping-pong [128, 1024] PSUM feed regions; ONE exp
    activation per <=1024-col segment minimizes the ACT engine's fixed
    per-instruction cost; rows wider than 1024 split into BALANCED halves
    so consecutive rows chase similar-sized exps through the ping-pong.  Feed slots are also "borrowed" (with the same
    parity rotation) as scratch psum for pair/permute/v work so producer
    chains don't serialize on the single work bank.
  * Software pipeline: scores+exp for row r are issued before attnV of row
    r-1; the causal diagonal is masked in place on the exp output (DVE
    2x-mode mult, Pool for head 3); 9-deep et tile rotation.
  * attnV accumulates out[i, v|den] slices in 3 persistent PSUM banks
    (65-wide slices; column 64 = softmax denominator via a ones-column in
    v).
  * o-projection streams during head 3; output splits: seq 0:1536 as
    [H, 1536] bf16 with 4-hc-batched [512,512] DMAs, seq 1536:1920 (query
    tiles 12-14, ready one attnV row before the end) and the final 128-wide
    tile-15 sliver as flat bf16 tensors so the kernel tail is two small
    transfers.  The host reassembles/stitches the three pieces.
"""

import numpy as np
import ml_dtypes

B, S, H = 2, 2048, 1024
NH, NKV, HD = 16, 4, 64
G = NH // NKV            # 4 q-heads per kv head
THETA = 10000.0
N_CORES = 8

BF16 = ml_dtypes.bfloat16

NSB = S // 512           # 4 chunks of 512
NST = S // 128           # 16 tiles of 128
KC = H // 128            # 8 contraction chunks
SCALE = float(1.0 / np.sqrt(HD))
N_WARM = 40              # PE p-state warmup matmuls (128 cols each)


LABEL_MAP = {}


def _build_nc():
    import concourse.mybir as mybir
    import concourse.tile as tile
    from concourse.tile import add_dep_helper
    from concourse import bacc

    f32 = mybir.dt.float32
    bf16 = mybir.dt.bfloat16

    nc = bacc.Bacc("TRN2", target_bir_lowering=False)

    # instruction-name -> source-op label, for the timeline analyzer
    LABEL_MAP.clear()
    _cur = ["init"]

    def L(s):
        _cur[0] = s

    _orig_gnin = nc.get_next_instruction_name

    def _gnin():
        nm = _orig_gnin()
        LABEL_MAP[nm] = _cur[0]
        return nm

    nc.get_next_instruction_name = _gnin

    xTd = nc.dram_tensor("xT", [H, S], bf16, kind="ExternalInput")
    wk8d = nc.dram_tensor("wkT8", [128, KC * 128], bf16, kind="ExternalInput")
    # q weights packed 2 heads per 128 columns (no RoPE duplication); the
    # per-head [q; q2] layout is recovered with one sign-permute matmul.
    wp0d = nc.dram_tensor("wp0T8", [128, KC * 128], bf16,
                          kind="ExternalInput")
    wp1d = nc.dram_tensor("wp1T8", [128, KC * 128], bf16,
                          kind="ExternalInput")
    pm0d = nc.dram_tensor("perm0", [128, 128], bf16, kind="ExternalInput")
    pm1d = nc.dram_tensor("perm1", [128, 128], bf16, kind="ExternalInput")
    wvd = nc.dram_tensor("wvT", [H, HD], bf16, kind="ExternalInput")
    csd = nc.dram_tensor("cs", [128, S], bf16, kind="ExternalInput")
    wod = nc.dram_tensor("woT", [G * HD, H], bf16, kind="ExternalInput")
    trid = nc.dram_tensor("trimask", [128, 128], bf16, kind="ExternalInput")
    djd = nc.dram_tensor("dupJ", [128, 128], bf16, kind="ExternalInput")
    idd = nc.dram_tensor("ident", [128, 128], bf16, kind="ExternalInput")
    # outputs: main seq cols 0:1536 as [H, 1536] bf16; seq 1536:2048 flat
    # bf16, per query tile ([feat%128, tile, half, hcg, s]) so each of the
    # last 4 query tiles streams out as soon as its attnV row completes.
    # The host reassembles.
    oTd = nc.dram_tensor("oT", [H, 3 * 512], bf16, kind="ExternalOutput")
    oT3d = nc.dram_tensor("oT3", [128, 2 * 4 * 384], bf16,
                          kind="ExternalOutput")
    oT2d = nc.dram_tensor("oT2", [128, 8 * 128], bf16, kind="ExternalOutput")

    with tile.TileContext(nc) as tc:
        import contextlib
        ctx = contextlib.ExitStack()
        with ctx:
            consts = ctx.enter_context(tc.tile_pool(name="consts", bufs=1))
            acts = ctx.enter_context(tc.tile_pool(name="acts", bufs=1))
            ep = ctx.enter_context(tc.tile_pool(name="exps", bufs=4))
            rcp = ctx.enter_context(tc.tile_pool(name="rcp", bufs=20))
            otp = ctx.enter_context(tc.tile_pool(name="ot", bufs=8))
            pav = ctx.enter_context(
                tc.tile_pool(name="ps_av", bufs=1, space="PSUM"))
            pfa = ctx.enter_context(
                tc.tile_pool(name="ps_fa", bufs=1, space="PSUM"))
            pfb = ctx.enter_context(
                tc.tile_pool(name="ps_fb", bufs=1, space="PSUM"))
            pw = ctx.enter_context(
                tc.tile_pool(name="ps_w", bufs=1, space="PSUM"))

            # ---- input DMAs, ordered by first use: k weights + first xt
            # column block feed the k/q0 projections; the remaining xt lands
            # column-major so qhat chunks stream in order.  wk/wq0 use
            # host-preswizzled contiguous [128, KC*128] layouts so their DMA
            # descriptors are 2KB (no sub-512B penalty), and the first xt
            # column block is split into kc-pair chunks so the k projection
            # can start after ~1/4 of it has landed.
            xt_sb = consts.tile([128, KC, S], bf16, tag="xt")
            wk_sb = consts.tile([128, KC * 128], bf16, tag="wk")
            wp_sb = [consts.tile([128, KC * 128], bf16, tag=f"wp{p}",
                                 name=f"wp{p}") for p in range(2)]
            pm_sb = [consts.tile([128, 128], bf16, tag=f"pm{p}",
                                 name=f"pm{p}") for p in range(2)]
            cs_sb = consts.tile([128, S], bf16, tag="cs")
            tri_sb = consts.tile([128, 128], bf16, tag="tri")
            dj_sb = consts.tile([128, 128], bf16, tag="dj")
            id_sb = consts.tile([128, 128], bf16, tag="id")
            wv_sb = consts.tile([128, KC, HD], bf16, tag="wv")
            wo_sb = consts.tile([128, 2, H], bf16, tag="wo")
            warm_sb = consts.tile([128, 128], bf16, tag="warm")

            # PE warmup: burn the p-state ramp against a memset tile while
            # the first input DMAs are in flight.
            nc.gpsimd.memset(warm_sb, 0.0)
            warm_ps = pw.tile([128, 128], f32, tag="w", name="warmps")
            for _ in range(N_WARM):
                nc.tensor.matmul(warm_ps, warm_sb, warm_sb,
                                 start=True, stop=True, skip_group_check=True)

            def xt_col(n):
                # two half-blocks so dependent projections can chase
                for half in range(2):
                    c = n * 512
                    r0 = half * 512
                    nc.sync.dma_start(
                        xt_sb[:, 4 * half:4 * half + 4, c:c + 512],
                        xTd[r0:r0 + 512, c:c + 512].rearrange(
                            "(kc p) m -> p kc m", p=128))

            # head 0 consumes key rows DESCENDING, so xt streams in reverse
            # column order: each arriving 512-col block unlocks the next 4
            # rows plus their k/v/q projection chunks.
            nc.sync.dma_start(wp_sb[0], wp0d[:, :])
            nc.sync.dma_start(wk_sb, wk8d[:, :])
            for kk in range(4):
                r0 = kk * 256
                nc.sync.dma_start(
                    xt_sb[:, 2 * kk:2 * kk + 2, 1536:2048],
                    xTd[r0:r0 + 256, 1536:2048].rearrange(
                        "(kc p) m -> p kc m", p=128))
                if kk == 0:
                    nc.sync.dma_start(pm_sb[0], pm0d[:, :])
                if kk == 1:
                    nc.sync.dma_start(
                        cs_sb[:, 1024:2048], csd[:, 1024:2048])
            nc.sync.dma_start(dj_sb, djd[:, :])
            nc.sync.dma_start(tri_sb, trid[:, :])
            nc.sync.dma_start(wv_sb, wvd.rearrange("(kc p) m -> p kc m", p=128))
            xt_col(2)
            nc.sync.dma_start(cs_sb[:, 0:1024], csd[:, 0:1024])
            xt_col(1)
            xt_col(0)
            nc.sync.dma_start(pm_sb[1], pm1d[:, :])
            nc.sync.dma_start(wp_sb[1], wp1d[:, :])
            nc.sync.dma_start(wo_sb, wod.rearrange("(kc p) m -> p kc m", p=128))
            nc.sync.dma_start(id_sb, idd[:, :])

            qhat = [acts.tile([128, S], bf16, tag=f"qh{m}", name=f"qhat{m}")
                    for m in range(G)]
            qpk = acts.tile([128, 2, S], bf16, tag="qpk")
            khat = acts.tile([128, S], bf16, tag="khat")
            ktmp = acts.tile([128, S], bf16, tag="ktmp")
            v_sb = acts.tile([128, NST, HD + 1], bf16, tag="vsb")
            attn_n = acts.tile([128, NST, G * HD], bf16, tag="attn")
            aT = acts.tile([128, 2, S], bf16, tag="aT")

            avb = [pav.tile([128, w], f32, tag=f"av{b}", name=f"avb{b}")
                   for b, w in ((0, 455), (1, 455), (2, 130))]

            def av_slice(it):
                b, o = it // 7, (it % 7) * 65
                return avb[b][:, o:o + 65]

            seg_counter = [0]

            def feed_tile(idx, ln):
                # ping-pong exp-feed regions, allocated per segment so the
                # pool slot rotation provides the WAR chain
                if idx % 2 == 0:
                    return pfa.tile([128, ln], f32, tag="fA", name="feed",
                                    padded_shape=[128, 1024])
                return pfb.tile([128, ln], f32, tag="fB", name="feed",
                                padded_shape=[128, 1024])

            def proj_psum(lhs_sb, n, ps):
                col = n * 512
                for kc in range(KC):
                    nc.tensor.matmul(
                        ps, lhs_sb[:, kc * 128:(kc + 1) * 128],
                        xt_sb[:, kc, col:col + 512],
                        start=(kc == 0), stop=(kc == KC - 1))

            def borrow_feed():
                # take the next feed slot out of the scores ping-pong; its
                # pool-rotation WAR lines up with the borrower's own deps
                ps = feed_tile(seg_counter[0], 512)
                seg_counter[0] += 1
                return ps

            def pair_proj(pr, n, ps=None):
                # raw packed projection of q-heads (2pr, 2pr+1); the psum is
                # copied to sbuf so the per-head permutes can stream from it
                L(f"pair_proj{pr}_{n}")
                if ps is None:
                    ps = pw.tile([128, 512], f32, tag="w", name="pspp")
                proj_psum(wp_sb[pr], n, ps)
                col = n * 512
                nc.vector.tensor_copy(qpk[:, pr, col:col + 512], ps)

            def q_permute(h, n, ps=None):
                # [q_h; sign-permuted q_h] via one 128x128 matmul, then the
                # RoPE cos/sin multiply
                L(f"q_permute{h}_{n}")
                col = n * 512
                if ps is None:
                    ps = pw.tile([128, 512], f32, tag="w", name="psqp")
                nc.tensor.matmul(ps, pm_sb[h % 2],
                                 qpk[:, h // 2, col:col + 512],
                                 start=True, stop=True)
                nc.vector.tensor_mul(
                    qhat[h][:, col:col + 512], ps, cs_sb[:, col:col + 512])

            def k_proj(n, ps=None):
                L(f"k_proj{n}")
                if ps is None:
                    ps = pw.tile([128, 512], f32, tag="w", name="psk")
                proj_psum(wk_sb, n, ps)
                col = n * 512
                nc.vector.tensor_mul(
                    ktmp[:, col:col + 512], ps, cs_sb[:, col:col + 512])

            def k_fold(n, psf=None, eng=0):
                L(f"k_fold{n}")
                col = n * 512
                if psf is None:
                    psf = pw.tile([128, 512], f32, tag="w", name="psf")
                nc.tensor.matmul(psf, dj_sb, ktmp[:, col:col + 512],
                                 start=True, stop=True)
                if eng == 0:
                    nc.vector.tensor_copy(khat[:, col:col + 512], psf)
                else:
                    nc.scalar.copy(khat[:, col:col + 512], psf)

            def k_chunk(n, ps=None, psf=None):
                k_proj(n, ps)
                k_fold(n, psf)

            def v_tiles(st0, nt, ps=None):
                L(f"v_tiles{st0}")
                # project nt seq-tiles of v through one work-psum residency
                if ps is None:
                    psv = pw.tile([128, nt, HD], f32, tag="w", name="psv",
                                  padded_shape=[128, 4, HD])
                else:
                    psv = ps.rearrange("p (t d) -> p t d", d=HD)[:, 0:nt, :]
                for t in range(nt):
                    st = st0 + t
                    for kc in range(KC):
                        nc.tensor.matmul(
                            psv[:, t, :],
                            xt_sb[:, kc, st * 128:(st + 1) * 128],
                            wv_sb[:, kc, :],
                            start=(t == 0 and kc == 0), stop=(kc == KC - 1),
                            skip_group_check=True)
                nc.vector.tensor_copy(
                    v_sb[:, st0:st0 + nt, 0:HD], psv)

            def transpose_tiles(hp, its):
                L(f"transp{hp}_{its[0]}")
                # slice transposes through one work-psum residency
                psx = pw.tile([128, len(its), 128], bf16, tag="w", name="pst",
                              padded_shape=[128, 4, 128])
                for t, it in enumerate(its):
                    nc.tensor.matmul(
                        psx[:, t, :], attn_n[:, it, hp * 128:(hp + 1) * 128],
                        id_sb, is_transpose=True,
                        start=(t == 0), stop=True, skip_group_check=True)
                c0 = its[0] * 128
                nc.vector.tensor_copy(
                    aT[:, hp, c0:c0 + len(its) * 128], psx)

            def transpose_group(hp, g4):
                transpose_tiles(hp, list(range(g4 * 4, g4 * 4 + 4)))

            # ---- prologue: chase the reverse-streamed xt column block 3.
            # k chunk 3 and pair0 chunk 3 interleave per-kc so both finish
            # right after the last xt3 sub-block lands.
            nc.gpsimd.memset(v_sb[:, :, HD:HD + 1], 1.0)
            L("k3+p03")
            psk3 = feed_tile(0, 512)
            psp3 = feed_tile(1, 512)
            for kc in range(KC):
                nc.tensor.matmul(
                    psk3, wk_sb[:, kc * 128:(kc + 1) * 128],
                    xt_sb[:, kc, 1536:2048],
                    start=(kc == 0), stop=(kc == KC - 1))
                nc.tensor.matmul(
                    psp3, wp_sb[0][:, kc * 128:(kc + 1) * 128],
                    xt_sb[:, kc, 1536:2048],
                    start=(kc == 0), stop=(kc == KC - 1))
            nc.vector.tensor_mul(ktmp[:, 1536:2048], psk3, cs_sb[:, 1536:2048])
            nc.scalar.copy(qpk[:, 0, 1536:2048], psp3)
            seg_counter[0] = 2
            k_fold(3, psf=pw.tile([128, 512], f32, tag="w", name="psf3"))
            q_permute(0, 3, ps=borrow_feed())

            def scores_row(h, jt, et, segs=None, cbs=None):
                L(f"scores{h}_{jt}")
                lo = jt * 128
                cols = S - lo
                lhsT = khat[:, lo:lo + 128]
                if segs is None:
                    if cols > 1024:
                        h1len = ((cols // 2 + 127) // 128) * 128
                        segs = [(lo, h1len), (lo + h1len, cols - h1len)]
                    else:
                        segs = [(lo, cols)]
                for si, (off, ln) in enumerate(segs):
                    region = feed_tile(seg_counter[0], ln)
                    seg_counter[0] += 1
                    done = 0
                    while done < ln:
                        cl = min(512, ln - done)
                        nc.tensor.matmul(
                            region[:, done:done + cl], lhsT,
                            qhat[h][:, off + done:off + done + cl],
                            start=True, stop=True)
                        done += cl
                    with tc.high_priority(offset=64):
                        nc.scalar.activation(
                            et[:, off:off + ln], region[:, 0:ln],
                            mybir.ActivationFunctionType.Exp, scale=SCALE)
                    if cbs is not None and si in cbs:
                        cbs[si]()
                # causal mask on diag tile: Pool, hidden by the pipeline
                L(f"mask{h}_{jt}")
                if h == 3:
                    nc.gpsimd.tensor_mul(et[:, lo:lo + 128],
                                         et[:, lo:lo + 128], tri_sb)
                else:
                    nc.vector.tensor_mul(et[:, lo:lo + 128],
                                         et[:, lo:lo + 128], tri_sb)

            def scores_pair(h, jt, et):
                # rows jt and jt+1 (combined cols <= 1024) share one feed
                # region and ONE exp; et is flat-packed: row jt at columns
                # [0, c1), row jt+1 at [c1, c1+c2)
                L(f"scores{h}_{jt}")
                lo1, lo2 = jt * 128, (jt + 1) * 128
                c1, c2 = S - lo1, S - lo2
                region = feed_tile(seg_counter[0], c1 + c2)
                seg_counter[0] += 1
                nc.tensor.matmul(region[:, 0:c1], khat[:, lo1:lo1 + 128],
                                 qhat[h][:, lo1:S], start=True, stop=True)
                nc.tensor.matmul(region[:, c1:c1 + c2],
                                 khat[:, lo2:lo2 + 128],
                                 qhat[h][:, lo2:S], start=True, stop=True)
                nc.scalar.activation(
                    et[:, 0:c1 + c2], region[:, 0:c1 + c2],
                    mybir.ActivationFunctionType.Exp, scale=SCALE)
                L(f"mask{h}_{jt}")
                if h == 3:
                    nc.gpsimd.tensor_mul(et[:, 0:128], et[:, 0:128], tri_sb)
                    nc.gpsimd.tensor_mul(et[:, c1:c1 + 128],
                                         et[:, c1:c1 + 128], tri_sb)
                else:
                    nc.vector.tensor_mul(et[:, 0:128], et[:, 0:128], tri_sb)
                    nc.vector.tensor_mul(et[:, c1:c1 + 128],
                                         et[:, c1:c1 + 128], tri_sb)
                return c1

            attnv_state = {}   # h -> bank_first dict

            def attnv_row(h, jt, et, desc=False, ebase=0):
                # In ascending key order slice jt is complete after this row
                # (stop + normalize); in descending order every slice stays
                # open until row 0 and normalization happens afterwards.
                # ebase shifts the et column origin for flat-packed rows.
                L(f"attnv{h}_{jt}")
                bank_first = attnv_state.setdefault(h, {})
                b1_hi = min(jt + 7, NST - 1)
                order = list(range(b1_hi, jt - 1, -1)) + \
                    list(range(NST - 1, b1_hi, -1))
                for it in order:
                    b = it // 7
                    first = b not in bank_first
                    c0 = it * 128 - ebase
                    mm = nc.tensor.matmul(
                        av_slice(it), et[:, c0:c0 + 128],
                        v_sb[:, jt, :],
                        start=first,
                        stop=(jt == 0 if desc else it == jt),
                        skip_group_check=True)
                    if first:
                        bank_first[b] = mm
                    elif jt == 0:
                        add_dep_helper(mm.ins, bank_first[b].ins,
                                       sync=False,
                                       reason="bank clear first")
                if not desc:
                    normalize_slice(h, jt)

            def normalize_slice(h, jt, eng=0, rc=None):
                L(f"norm{h}_{jt}")
                pso = av_slice(jt)
                if rc is None:
                    rc = rcp.tile([128, 1], f32, tag="rc", name="rc")
                    nc.vector.reciprocal(rc, pso[:, HD:HD + 1])
                if eng == 0:
                    nc.vector.tensor_scalar_mul(
                        attn_n[:, jt, h * HD:(h + 1) * HD], pso[:, 0:HD], rc)
                else:
                    nc.scalar.mul(
                        attn_n[:, jt, h * HD:(h + 1) * HD], pso[:, 0:HD], rc)

            # oproj -------------------------------------------------------
            # Column groups g=0..2 keep the original 4-hc-batched [512,512]
            # output DMAs (few HWDGE entries).  Group 3 (seq 1536:2048) is
            # split: a 384-wide part (query tiles 12-14, ready one attnV row
            # before the end) drained + DMA'd flat, and a final 128-wide
            # sliver (tile 15) that is DMA'd directly from PSUM as f32 so
            # the kernel tail is one small transfer with no drain wait.
            oproj_pending = [(g, hc) for g in range(3) for hc in range(KC)]
            ot_state = {}

            def oproj_chunk(ps, drain_eng):
                g, hc = oproj_pending.pop(0)
                L(f"oproj{g}_{hc}")
                col = g * 512
                for kc2 in range(2):
                    nc.tensor.matmul(
                        ps, wo_sb[:, kc2, hc * 128:(hc + 1) * 128],
                        aT[:, kc2, col:col + 512],
                        start=(kc2 == 0), stop=(kc2 == 1))
                if hc % 4 == 0:
                    ot_state[g] = otp.tile([128, 4, 512], bf16, tag="otb",
                                           name="otb")
                ot = ot_state[g]
                with tc.high_priority(offset=-64):
                    if drain_eng == 0:
                        nc.vector.tensor_copy(ot[:, hc % 4, :], ps)
                    elif drain_eng == 1:
                        nc.scalar.copy(ot[:, hc % 4, :], ps)
                    else:
                        nc.vector.tensor_copy(ot[:, hc % 4, 0:256],
                                              ps[:, 0:256])
                        nc.scalar.copy(ot[:, hc % 4, 256:512],
                                       ps[:, 256:512])
                if hc % 4 == 3:
                    r0 = (hc // 4) * 512
                    nc.sync.dma_start(
                        oTd[r0:r0 + 512, col:col + 512].rearrange(
                            "(c p) m -> p c m", p=128), ot)

            g3_pending = list(range(KC))
            ot3_state = {}

            def g3_chunk(ps, drain_eng):
                hc = g3_pending.pop(0)
                L(f"g3_{hc}")
                for kc2 in range(2):
                    nc.tensor.matmul(
                        ps[:, 0:384], wo_sb[:, kc2, hc * 128:(hc + 1) * 128],
                        aT[:, kc2, 1536:1920],
                        start=(kc2 == 0), stop=(kc2 == 1))
                if hc % 4 == 0:
                    ot3_state[hc // 4] = otp.tile(
                        [128, 4, 384], bf16, tag="ot3", name="ot3")
                ot = ot3_state[hc // 4]
                if drain_eng == 0:
                    nc.vector.tensor_copy(ot[:, hc % 4, :], ps[:, 0:384])
                else:
                    nc.scalar.copy(ot[:, hc % 4, :], ps[:, 0:384])
                if hc % 4 == 3:
                    grp = hc // 4
                    nc.sync.dma_start(
                        oT3d[:, grp * 1536:(grp + 1) * 1536], ot)

            # ---- main pipelined loop ------------------------------------
            # head 0 walks key rows DESCENDING (matched to the reverse xt
            # stream: each xt column block unlocks 4 more rows and their
            # k/v/pair-projection chunks); heads 1-3 ascend as before.
            seq = [(0, jt) for jt in range(NST - 1, -1, -1)] + \
                [(h, jt) for h in range(1, G) for jt in range(NST)]
            prev = None
            pair_state = {}
            for (h, jt) in seq:
                if h > 0 and jt in (12, 14):
                    # rows (12,13) and (14,15) share one exp instruction
                    w12 = 2 * S - (2 * jt + 1) * 128
                    et = ep.tile([128, w12], bf16, tag="e",
                                 name=f"e{h}_{jt}")
                    pair_state = {"et": et, "c1": scores_pair(h, jt, et)}
                    ebase = jt * 128
                elif h > 0 and jt in (13, 15):
                    et = pair_state["et"]
                    ebase = jt * 128 - pair_state["c1"]
                else:
                    et = ep.tile([128, S], bf16, tag="e", name=f"e{h}_{jt}")
                    scores_row(h, jt, et)
                    ebase = 0
                if prev is not None:
                    attnv_row(prev[0], prev[1], prev[2], desc=(prev[0] == 0),
                              ebase=prev[3])
                    if prev[:2] == (0, 0):
                        # head 0 ran descending: all 16 attnV slices close
                        # at row 0; normalize split across DVE/ACT, in
                        # descending slice order to pipeline with head 1's
                        # bank-clearing attnV writes (slice 15 first).
                        # Reciprocals batch first so the ACT muls never wait
                        # on an individual DVE recip.
                        rcs = {}
                        for i in range(NST - 1, -1, -1):
                            rcs[i] = rcp.tile([128, 1], f32, tag="rc",
                                              name="rcb")
                            nc.vector.reciprocal(
                                rcs[i], av_slice(i)[:, HD:HD + 1])
                        for i in range(NST - 1, -1, -1):
                            normalize_slice(0, i, eng=0, rc=rcs[i])
                prev = (h, jt, et, ebase)

                # interleaved producer work; head-0 rows chase the reverse
                # xt stream, so the projection bursts sit right after the
                # last row that only needs already-landed data.
                if h == 0:
                    if jt == 15:
                        v_tiles(14, 2, ps=borrow_feed())
                    if jt == 14:
                        v_tiles(12, 2, ps=borrow_feed())
                    if jt == 13:
                        k_proj(2)
                    if jt == 12:
                        k_fold(2)
                        pair_proj(0, 2, ps=borrow_feed())
                        q_permute(0, 2, ps=borrow_feed())
                    if jt == 11:
                        v_tiles(8, 4, ps=borrow_feed())
                    if jt == 9:
                        k_proj(1)
                    if jt == 8:
                        k_fold(1)
                        pair_proj(0, 1, ps=borrow_feed())
                        q_permute(0, 1, ps=borrow_feed())
                    if jt == 7:
                        v_tiles(4, 4)
                    if jt == 5:
                        k_proj(0)
                    if jt == 4:
                        k_fold(0)
                        pair_proj(0, 0, ps=borrow_feed())
                        q_permute(0, 0, ps=borrow_feed())
                    if jt == 3:
                        v_tiles(0, 4)
                    if jt in (3, 2, 1, 0):
                        q_permute(1, 3 - jt)
                if h == 1:
                    if jt in (1, 4, 7, 10):
                        pair_proj(1, (jt - 1) // 3)
                    if jt in (2, 5, 8, 11):
                        q_permute(2, (jt - 2) // 3)
                if h == 2 and jt in (1, 4, 7, 10):
                    q_permute(3, (jt - 1) // 3)
                if h == 2 and jt in (3, 7, 11, 15):
                    transpose_group(0, jt // 4)
                if h == 3:
                    if jt in (5, 9, 13):
                        transpose_group(1, (jt - 5) // 4)
                    if jt >= 5 and oproj_pending and \
                            oproj_pending[0][0] * 4 + 5 <= jt:
                        ps = pw.tile([128, 512], f32, tag="w", name="psow")
                        oproj_chunk(ps, drain_eng=0)
                    if jt >= 8 and oproj_pending and \
                            oproj_pending[0][0] * 4 + 5 <= jt:
                        ps = pav.tile([128, 512], f32, tag="av0",
                                      name="psoa")
                        oproj_chunk(ps, drain_eng=1 if jt >= 12 else 0)
                    if jt in (11, 13, 15) and oproj_pending and \
                            oproj_pending[0][0] * 4 + 5 <= jt:
                        ps = pav.tile([128, 512], f32, tag="av0",
                                      name="psoa2")
                        oproj_chunk(ps, drain_eng=1)
                    if jt in (14, 15) and oproj_pending and \
                            oproj_pending[0][0] * 4 + 5 <= jt:
                        ps = pav.tile([128, 512], f32, tag="av1",
                                      name="psob")
                        oproj_chunk(ps, drain_eng=1)
                    if jt == 15:
                        # rows 12-14 of head 3 are normalized; pair-1
                        # transposes for tiles 12-14 unblock the 384-wide
                        # part of column group 3.
                        transpose_tiles(1, [12, 13, 14])

            # flush: last attnV row, then the 384-wide part of group 3
            # (query tiles 12-14), the tile-15 transpose, and the sliver.
            attnv_row(prev[0], prev[1], prev[2], ebase=prev[3])
            ti = 0
            slots = ["fA", "fB", "av0", "w", "av1"]
            pools = {"w": pw, "fA": pfa, "fB": pfb, "av0": pav, "av1": pav}
            while oproj_pending:
                tag = slots[ti % len(slots)]
                ps = pools[tag].tile([128, 512], f32, tag=tag, name="psot")
                oproj_chunk(ps, drain_eng=ti % 2)
                ti += 1

            def g3_next(drain_eng):
                tag = slots[ti % len(slots)]
                ps = pools[tag].tile([128, 512], f32, tag=tag, name="psog3")
                g3_chunk(ps, drain_eng)

            # two g3 chunks cover the normalize latency of row 15, then the
            # tile-15 transpose slots in, then the rest.
            g3_next(0)
            g3_next(1)
            ti += 2
            transpose_tiles(1, [15])
            while g3_pending:
                g3_next(ti % 2)
                ti += 1
            del g3_next

            # sliver: 8 feature chunks x 128 seq cols; two [128, 512] f32
            # psum tiles, drained on parallel engines, two small flat DMAs.
            for half in range(2):
                L(f"sliver{half}")
                psl = (pfa if half == 0 else pfb).tile(
                    [128, 512], f32, tag=("fA" if half == 0 else "fB"),
                    name="psliv", padded_shape=[128, 1024])
                for sub in range(4):
                    hc = half * 4 + sub
                    for kc2 in range(2):
                        nc.tensor.matmul(
                            psl[:, sub * 128:(sub + 1) * 128],
                            wo_sb[:, kc2, hc * 128:(hc + 1) * 128],
                            aT[:, kc2, 1920:2048],
                            start=(kc2 == 0), stop=(kc2 == 1))
                ot2 = otp.tile([128, 512], bf16, tag="ot2", name="ot2")
                if half == 0:
                    nc.vector.tensor_copy(ot2, psl)
                else:
                    nc.scalar.copy(ot2, psl)
                nc.sync.dma_start(
                    oT2d[:, half * 512:(half + 1) * 512], ot2)

    nc.finalize()
    return nc


def _host_inputs(hidden_states, position_ids, wq, wk, wv, wo):
    """Build the 8 per-core input maps."""
    def w2_of(w):
        # w: [64, H] rows of one head; returns sign-permuted rows
        w2 = np.empty_like(w)
        w2[:32] = -w[32:64]
        w2[32:] = w[:32]
        return w2

    trimask = np.triu(np.ones((128, 128), np.float32)).astype(BF16)
    dupJ = np.zeros((128, 128), np.float32)
    for p in range(128):
        dupJ[p, p % 64] = 1.0
        dupJ[p, p % 64 + 64] = 1.0
    dupJ = dupJ.astype(BF16)
    ident = np.eye(128, dtype=np.float32).astype(BF16)

    # perm[lh][k, p]: rows of the packed pair projection (head lh occupies
    # input rows 64*lh..64*lh+63) -> [q; sign-permuted q] output rows
    perms = []
    for lh in range(2):
        P = np.zeros((128, 128), np.float32)
        b = 64 * lh
        for p in range(64):
            P[b + p, p] = 1.0
        for i in range(32):
            P[b + 32 + i, 64 + i] = -1.0
            P[b + i, 96 + i] = 1.0
        perms.append(P.astype(BF16))

    def swz(wT):
        # [H, 128] -> [128, KC*128] with 2KB-contiguous DMA descriptors
        return np.ascontiguousarray(
            wT.reshape(KC, 128, 128).transpose(1, 0, 2)
            .reshape(128, KC * 128)).astype(BF16)

    in_maps = []
    for core in range(N_CORES):
        b, kv = core // NKV, core % NKV
        xT = np.ascontiguousarray(hidden_states[b].T).astype(BF16)

        wkh = wk[kv * HD:(kv + 1) * HD]
        wkT8 = swz(np.concatenate([wkh.T, w2_of(wkh).T], axis=1))
        wpT8 = []
        for pr in range(2):
            h0 = kv * G + 2 * pr
            wpT8.append(swz(wq[h0 * HD:(h0 + 2) * HD].T))

        wvT = np.ascontiguousarray(wv[kv * HD:(kv + 1) * HD].T).astype(BF16)
        woT = np.ascontiguousarray(
            wo[:, kv * G * HD:(kv + 1) * G * HD].T).astype(BF16)

        inv = 1.0 / (THETA ** (np.arange(0, HD, 2, dtype=np.float32) / HD))
        freqs = position_ids[b].astype(np.float32)[:, None] * inv[None, :]
        emb = np.concatenate([freqs, freqs], axis=-1)       # [S, 64]
        cs = np.concatenate([np.cos(emb).T, np.sin(emb).T], axis=0)  # [128, S]
        cs = np.ascontiguousarray(cs).astype(BF16)

        in_maps.append({
            "xT": xT, "wvT": wvT, "cs": cs, "woT": woT,
            "trimask": trimask, "dupJ": dupJ, "ident": ident,
            "wkT8": wkT8, "wp0T8": wpT8[0], "wp1T8": wpT8[1],
            "perm0": perms[0], "perm1": perms[1],
        })
    return in_maps


_NC_CACHE = {}


def run_cores(in_maps, trace=False, trace_kwargs=None):
    from concourse.bass_utils import run_bass_kernel_spmd
    if "nc" not in _NC_CACHE:
        _NC_CACHE["nc"] = _build_nc()
    nc = _NC_CACHE["nc"]
    return run_bass_kernel_spmd(
        nc, in_maps, core_ids=list(range(N_CORES)),
        trace=trace, **(trace_kwargs or {}))


def assemble(res):
    """Gather the 3 per-core output pieces into the full [B, S, H] output."""
    out = np.zeros((B, S, H), np.float32)
    for core in range(N_CORES):
        b = core // NKV
        r = res.results[core]
        out[b, 0:1536] += r["oT"].T.astype(np.float32)
        out[b, 1536:1920] += (
            r["oT3"].reshape(128, 2, 4, 384).transpose(3, 1, 2, 0)
            .reshape(384, H).astype(np.float32))
        out[b, 1920:2048] += (
            r["oT2"].reshape(128, 8, 128).transpose(2, 1, 0)
            .reshape(128, H).astype(np.float32))
    return out


def kernel(hidden_states, attention_mask, position_ids, wq, wk, wv, wo):
    hidden_states = np.asarray(hidden_states, dtype=np.float32)
    position_ids = np.asarray(position_ids)
    wq = np.asarray(wq, dtype=np.float32)
    wk = np.asarray(wk, dtype=np.float32)
    wv = np.asarray(wv, dtype=np.float32)
    wo = np.asarray(wo, dtype=np.float32)

    in_maps = _host_inputs(hidden_states, position_ids, wq, wk, wv, wo)
    res = run_cores(in_maps)
    return assemble(res)



# revision 133
# speedup vs baseline: 1.0093x; 1.0013x over previous
"""Trainium2 Bass kernel for MimiAttention (GQA + RoPE + causal softmax).

Problem: B=2, S=2048, H=1024, NH=16 q-heads, NKV=4 kv-heads, HD=64.
Sharding: 8 cores = 2 (batch) x 4 (kv-group).  Each core computes one batch's
attention for one GQA group (4 q-heads sharing 1 kv head) and the partial
o-projection for those heads; the host sums the 4 partials per batch.

Design (all matmuls bf16 with fp32 psum; ~127us cost-model time per core):
  * Packed q-projections: q heads are projected 2-at-a-time with UNduplicated
    weights ([q_h; q_h+1] on 128 psum rows), then each head's RoPE layout
    [q; q2] (q2 = sign-permuted rows) is recovered with a single 128x128
    permute matmul and multiplied by cs = [cos; sin].  This cuts q-projection
    PE cycles ~1.6x vs. duplicated-weight columns.  khat = [k_rot; k_rot]
    via the J-fold matmul as before.
  * PE p-state warmup: ~44 matmuls against a memset tile burn the tensor
    engine's slow-clock ramp while the first input DMAs are in flight.
  * Head 0 walks key rows DESCENDING while xt streams column-blocks in
    reverse (block 3 first, kc-half granularity): row jt only needs xt
    columns >= 128*jt, so each arriving block unlocks the next 4 rows plus
    their k/v/pair-projection chunks and the PE never waits long for data.
    All 16 attnV slices then close at row 0 and are normalized in one burst
    (reciprocals batched first, then multiplies, all on DVE, descending
    slice order to pipeline with head 1's bank-clearing writes).  Heads 1-3
    ascend, with per-row slice normalization as usual.
  * Scores computed transposed (scoresT[j, i]) one key-tile row at a time,
    streamed through two ping-pong [128, 1024] PSUM feed regions; ONE exp
    activation per <=1024-col segment minimizes the ACT engine's fixed
    per-instruction cost; rows wider than 1024 split into BALANCED halves
    so consecutive rows chase similar-sized exps through the ping-pong.  Feed slots are also "borrowed" (with the same
    parity rotation) as scratch psum for pair/permute/v work so producer
    chains don't serialize on the single work bank.
  * Software pipeline: scores+exp for row r are issued before attnV of row
    r-1; the causal diagonal is masked in place on the exp output (DVE
    2x-mode mult, Pool for head 3); 9-deep et tile rotation.
  * attnV accumulates out[i, v|den] slices in 3 persistent PSUM banks
    (65-wide slices; column 64 = softmax denominator via a ones-column in
    v).
  * o-projection streams during head 3; output splits: seq 0:1536 as
    [H, 1536] bf16 with 4-hc-batched [512,512] DMAs, seq 1536:1920 (query
    tiles 12-14, ready one attnV row before the end) and the final 128-wide
    tile-15 sliver as flat bf16 tensors so the kernel tail is two small
    transfers.  The host reassembles/stitches the three pieces.
"""

import numpy as np
import ml_dtypes

B, S, H = 2, 2048, 1024
NH, NKV, HD = 16, 4, 64
G = NH // NKV            # 4 q-heads per kv head
THETA = 10000.0
N_CORES = 8

BF16 = ml_dtypes.bfloat16

NSB = S // 512           # 4 chunks of 512
NST = S // 128           # 16 tiles of 128
KC = H // 128            # 8 contraction chunks
SCALE = float(1.0 / np.sqrt(HD))
N_WARM = 40              # PE p-state warmup matmuls (128 cols each)


LABEL_MAP = {}


def _build_nc():
    import concourse.mybir as mybir
    import concourse.tile as tile
    from concourse.tile import add_dep_helper
    from concourse import bacc

    f32 = mybir.dt.float32
    bf16 = mybir.dt.bfloat16

    nc = bacc.Bacc("TRN2", target_bir_lowering=False)

    # instruction-name -> source-op label, for the timeline analyzer
    LABEL_MAP.clear()
    _cur = ["init"]

    def L(s):
        _cur[0] = s

    _orig_gnin = nc.get_next_instruction_name

    def _gnin():
        nm = _orig_gnin()
        LABEL_MAP[nm] = _cur[0]
        return nm

    nc.get_next_instruction_name = _gnin

    xTd = nc.dram_tensor("xT", [H, S], bf16, kind="ExternalInput")
    wk8d = nc.dram_tensor("wkT8", [128, KC * 128], bf16, kind="ExternalInput")
    # q weights packed 2 heads per 128 columns (no RoPE duplication); the
    # per-head [q; q2] layout is recovered with one sign-permute matmul.
    wp0d = nc.dram_tensor("wp0T8", [128, KC * 128], bf16,
                          kind="ExternalInput")
    wp1d = nc.dram_tensor("wp1T8", [128, KC * 128], bf16,
                          kind="ExternalInput")
    pm0d = nc.dram_tensor("perm0", [128, 128], bf16, kind="ExternalInput")
    pm1d = nc.dram_tensor("perm1", [128, 128], bf16, kind="ExternalInput")
    wvd = nc.dram_tensor("wvT", [H, HD], bf16, kind="ExternalInput")
    csd = nc.dram_tensor("cs", [128, S], bf16, kind="ExternalInput")
    wod = nc.dram_tensor("woT", [G * HD, H], bf16, kind="ExternalInput")
    trid = nc.dram_tensor("trimask", [128, 128], bf16, kind="ExternalInput")
    djd = nc.dram_tensor("dupJ", [128, 128], bf16, kind="ExternalInput")
    idd = nc.dram_tensor("ident", [128, 128], bf16, kind="ExternalInput")
    # outputs: main seq cols 0:1536 as [H, 1536] bf16; seq 1536:2048 flat
    # bf16, per query tile ([feat%128, tile, half, hcg, s]) so each of the
    # last 4 query tiles streams out as soon as its attnV row completes.
    # The host reassembles.
    oTd = nc.dram_tensor("oT", [H, 3 * 512], bf16, kind="ExternalOutput")
    oT3d = nc.dram_tensor("oT3", [128, 2 * 4 * 384], bf16,
                          kind="ExternalOutput")
    oT2d = nc.dram_tensor("oT2", [128, 8 * 128], bf16, kind="ExternalOutput")

    with tile.TileContext(nc) as tc:
        import contextlib
        ctx = contextlib.ExitStack()
        with ctx:
            consts = ctx.enter_context(tc.tile_pool(name="consts", bufs=1))
            acts = ctx.enter_context(tc.tile_pool(name="acts", bufs=1))
            ep = ctx.enter_context(tc.tile_pool(name="exps", bufs=4))
            rcp = ctx.enter_context(tc.tile_pool(name="rcp", bufs=20))
            otp = ctx.enter_context(tc.tile_pool(name="ot", bufs=8))
            pav = ctx.enter_context(
                tc.tile_pool(name="ps_av", bufs=1, space="PSUM"))
            pfa = ctx.enter_context(
                tc.tile_pool(name="ps_fa", bufs=1, space="PSUM"))
            pfb = ctx.enter_context(
                tc.tile_pool(name="ps_fb", bufs=1, space="PSUM"))
            pw = ctx.enter_context(
                tc.tile_pool(name="ps_w", bufs=1, space="PSUM"))

            # ---- input DMAs, ordered by first use: k weights + first xt
            # column block feed the k/q0 projections; the remaining xt lands
            # column-major so qhat chunks stream in order.  wk/wq0 use
            # host-preswizzled contiguous [128, KC*128] layouts so their DMA
            # descriptors are 2KB (no sub-512B penalty), and the first xt
            # column block is split into kc-pair chunks so the k projection
            # can start after ~1/4 of it has landed.
            xt_sb = consts.tile([128, KC, S], bf16, tag="xt")
            wk_sb = consts.tile([128, KC * 128], bf16, tag="wk")
            wp_sb = [consts.tile([128, KC * 128], bf16, tag=f"wp{p}",
                                 name=f"wp{p}") for p in range(2)]
            pm_sb = [consts.tile([128, 128], bf16, tag=f"pm{p}",
                                 name=f"pm{p}") for p in range(2)]
            cs_sb = consts.tile([128, S], bf16, tag="cs")
            tri_sb = consts.tile([128, 128], bf16, tag="tri")
            dj_sb = consts.tile([128, 128], bf16, tag="dj")
            id_sb = consts.tile([128, 128], bf16, tag="id")
            wv_sb = consts.tile([128, KC, HD], bf16, tag="wv")
            wo_sb = consts.tile([128, 2, H], bf16, tag="wo")
            warm_sb = consts.tile([128, 128], bf16, tag="warm")

            # PE warmup: burn the p-state ramp against a memset tile while
            # the first input DMAs are in flight.
            nc.gpsimd.memset(warm_sb, 0.0)
            warm_ps = pw.tile([128, 128], f32, tag="w", name="warmps")
            for _ in range(N_WARM):
                nc.tensor.matmul(warm_ps, warm_sb, warm_sb,
                                 start=True, stop=True, skip_group_check=True)

            def xt_col(n):
                # two half-blocks so dependent projections can chase
                for half in range(2):
                    c = n * 512
                    r0 = half * 512
                    nc.sync.dma_start(
                        xt_sb[:, 4 * half:4 * half + 4, c:c + 512],
                        xTd[r0:r0 + 512, c:c + 512].rearrange(
                            "(kc p) m -> p kc m", p=128))

            # head 0 consumes key rows DESCENDING, so xt streams in reverse
            # column order: each arriving 512-col block unlocks the next 4
            # rows plus their k/v/q projection chunks.
            nc.sync.dma_start(wp_sb[0], wp0d[:, :])
            nc.sync.dma_start(wk_sb, wk8d[:, :])
            for kk in range(4):
                r0 = kk * 256
                nc.sync.dma_start(
                    xt_sb[:, 2 * kk:2 * kk + 2, 1536:2048],
                    xTd[r0:r0 + 256, 1536:2048].rearrange(
                        "(kc p) m -> p kc m", p=128))
                if kk == 0:
                    nc.sync.dma_start(pm_sb[0], pm0d[:, :])
                if kk == 1:
                    nc.sync.dma_start(
                        cs_sb[:, 1024:2048], csd[:, 1024:2048])
            nc.sync.dma_start(dj_sb, djd[:, :])
            nc.sync.dma_start(tri_sb, trid[:, :])
            nc.sync.dma_start(wv_sb, wvd.rearrange("(kc p) m -> p kc m", p=128))
            xt_col(2)
            nc.sync.dma_start(cs_sb[:, 0:1024], csd[:, 0:1024])
            xt_col(1)
            xt_col(0)
            nc.sync.dma_start(pm_sb[1], pm1d[:, :])
            nc.sync.dma_start(wp_sb[1], wp1d[:, :])
            nc.sync.dma_start(wo_sb, wod.rearrange("(kc p) m -> p kc m", p=128))
            nc.sync.dma_start(id_sb, idd[:, :])

            qhat = [acts.tile([128, S], bf16, tag=f"qh{m}", name=f"qhat{m}")
                    for m in range(G)]
            qpk = acts.tile([128, 2, S], bf16, tag="qpk")
            khat = acts.tile([128, S], bf16, tag="khat")
            ktmp = acts.tile([128, S], bf16, tag="ktmp")
            v_sb = acts.tile([128, NST, HD + 1], bf16, tag="vsb")
            attn_n = acts.tile([128, NST, G * HD], bf16, tag="attn")
            aT = acts.tile([128, 2, S], bf16, tag="aT")

            avb = [pav.tile([128, w], f32, tag=f"av{b}", name=f"avb{b}")
                   for b, w in ((0, 455), (1, 455), (2, 130))]

            def av_slice(it):
                b, o = it // 7, (it % 7) * 65
                return avb[b][:, o:o + 65]

            seg_counter = [0]

            def feed_tile(idx, ln):
                # ping-pong exp-feed regions, allocated per segment so the
                # pool slot rotation provides the WAR chain
                if idx % 2 == 0:
                    return pfa.tile([128, ln], f32, tag="fA", name="feed",
                                    padded_shape=[128, 1024])
                return pfb.tile([128, ln], f32, tag="fB", name="feed",
                                padded_shape=[128, 1024])

            def proj_psum(lhs_sb, n, ps):
                col = n * 512
                for kc in range(KC):
                    nc.tensor.matmul(
                        ps, lhs_sb[:, kc * 128:(kc + 1) * 128],
                        xt_sb[:, kc, col:col + 512],
                        start=(kc == 0), stop=(kc == KC - 1))

            def borrow_feed():
                # take the next feed slot out of the scores ping-pong; its
                # pool-rotation WAR lines up with the borrower's own deps
                ps = feed_tile(seg_counter[0], 512)
                seg_counter[0] += 1
                return ps

            def pair_proj(pr, n, ps=None):
                # raw packed projection of q-heads (2pr, 2pr+1); the psum is
                # copied to sbuf so the per-head permutes can stream from it
                L(f"pair_proj{pr}_{n}")
                if ps is None:
                    ps = pw.tile([128, 512], f32, tag="w", name="pspp")
                proj_psum(wp_sb[pr], n, ps)
                col = n * 512
                nc.vector.tensor_copy(qpk[:, pr, col:col + 512], ps)

            def q_permute(h, n, ps=None):
                # [q_h; sign-permuted q_h] via one 128x128 matmul, then the
                # RoPE cos/sin multiply
                L(f"q_permute{h}_{n}")
                col = n * 512
                if ps is None:
                    ps = pw.tile([128, 512], f32, tag="w", name="psqp")
                nc.tensor.matmul(ps, pm_sb[h % 2],
                                 qpk[:, h // 2, col:col + 512],
                                 start=True, stop=True)
                nc.vector.tensor_mul(
                    qhat[h][:, col:col + 512], ps, cs_sb[:, col:col + 512])

            def k_proj(n, ps=None):
                L(f"k_proj{n}")
                if ps is None:
                    ps = pw.tile([128, 512], f32, tag="w", name="psk")
                proj_psum(wk_sb, n, ps)
                col = n * 512
                nc.vector.tensor_mul(
                    ktmp[:, col:col + 512], ps, cs_sb[:, col:col + 512])

            def k_fold(n, psf=None, eng=0):
                L(f"k_fold{n}")
                col = n * 512
                if psf is None:
                    psf = pw.tile([128, 512], f32, tag="w", name="psf")
                nc.tensor.matmul(psf, dj_sb, ktmp[:, col:col + 512],
                                 start=True, stop=True)
                if eng == 0:
                    nc.vector.tensor_copy(khat[:, col:col + 512], psf)
                else:
                    nc.scalar.copy(khat[:, col:col + 512], psf)

            def k_chunk(n, ps=None, psf=None):
                k_proj(n, ps)
                k_fold(n, psf)

            def v_tiles(st0, nt, ps=None):
                L(f"v_tiles{st0}")
                # project nt seq-tiles of v through one work-psum residency
                if ps is None:
                    psv = pw.tile([128, nt, HD], f32, tag="w", name="psv",
                                  padded_shape=[128, 4, HD])
                else:
                    psv = ps.rearrange("p (t d) -> p t d", d=HD)[:, 0:nt, :]
                for t in range(nt):
                    st = st0 + t
                    for kc in range(KC):
                        nc.tensor.matmul(
                            psv[:, t, :],
                            xt_sb[:, kc, st * 128:(st + 1) * 128],
                            wv_sb[:, kc, :],
                            start=(t == 0 and kc == 0), stop=(kc == KC - 1),
                            skip_group_check=True)
                nc.vector.tensor_copy(
                    v_sb[:, st0:st0 + nt, 0:HD], psv)

            def transpose_tiles(hp, its):
                L(f"transp{hp}_{its[0]}")
                # slice transposes through one work-psum residency
                psx = pw.tile([128, len(its), 128], bf16, tag="w", name="pst",
                              padded_shape=[128, 4, 128])
                for t, it in enumerate(its):
                    nc.tensor.matmul(
                        psx[:, t, :], attn_n[:, it, hp * 128:(hp + 1) * 128],
                        id_sb, is_transpose=True,
                        start=(t == 0), stop=True, skip_group_check=True)
                c0 = its[0] * 128
                nc.vector.tensor_copy(
                    aT[:, hp, c0:c0 + len(its) * 128], psx)

            def transpose_group(hp, g4):
                transpose_tiles(hp, list(range(g4 * 4, g4 * 4 + 4)))

            # ---- prologue: chase the reverse-streamed xt column block 3.
            # k chunk 3 and pair0 chunk 3 interleave per-kc so both finish
            # right after the last xt3 sub-block lands.
            nc.gpsimd.memset(v_sb[:, :, HD:HD + 1], 1.0)
            L("k3+p03")
            psk3 = feed_tile(0, 512)
            psp3 = feed_tile(1, 512)
            for kc in range(KC):
                nc.tensor.matmul(
                    psk3, wk_sb[:, kc * 128:(kc + 1) * 128],
                    xt_sb[:, kc, 1536:2048],
                    start=(kc == 0), stop=(kc == KC - 1))
                nc.tensor.matmul(
                    psp3, wp_sb[0][:, kc * 128:(kc + 1) * 128],
                    xt_sb[:, kc, 1536:2048],
                    start=(kc == 0), stop=(kc == KC - 1))
            nc.vector.tensor_mul(ktmp[:, 1536:2048], psk3, cs_sb[:, 1536:2048])
            nc.scalar.copy(qpk[:, 0, 1536:2048], psp3)
            seg_counter[0] = 2
            k_fold(3, psf=pw.tile([128, 512], f32, tag="w", name="psf3"))
            q_permute(0, 3, ps=borrow_feed())

            def scores_row(h, jt, et, segs=None, cbs=None):
                L(f"scores{h}_{jt}")
                lo = jt * 128
                cols = S - lo
                lhsT = khat[:, lo:lo + 128]
                if segs is None:
                    if cols > 1024:
                        h1len = ((cols // 2 + 127) // 128) * 128
                        segs = [(lo, h1len), (lo + h1len, cols - h1len)]
                    else:
                        segs = [(lo, cols)]
                for si, (off, ln) in enumerate(segs):
                    region = feed_tile(seg_counter[0], ln)
                    seg_counter[0] += 1
                    done = 0
                    while done < ln:
                        cl = min(512, ln - done)
                        nc.tensor.matmul(
                            region[:, done:done + cl], lhsT,
                            qhat[h][:, off + done:off + done + cl],
                            start=True, stop=True)
                        done += cl
                    with tc.high_priority(offset=64):
                        nc.scalar.activation(
                            et[:, off:off + ln], region[:, 0:ln],
                            mybir.ActivationFunctionType.Exp, scale=SCALE)
                    if cbs is not None and si in cbs:
                        cbs[si]()
                # causal mask on diag tile: Pool, hidden by the pipeline
                L(f"mask{h}_{jt}")
                if h == 3:
                    nc.gpsimd.tensor_mul(et[:, lo:lo + 128],
                                         et[:, lo:lo + 128], tri_sb)
                else:
                    nc.vector.tensor_mul(et[:, lo:lo + 128],
                                         et[:, lo:lo + 128], tri_sb)

            def scores_pair(h, ra, rb, et):
                # rows ra and rb (combined cols <= 1024) share one feed
                # region and ONE exp; et is flat-packed: row ra at columns
                # [0, c1), row rb at [c1, c1+c2)
                L(f"scores{h}_{ra}")
                lo1, lo2 = ra * 128, rb * 128
                c1, c2 = S - lo1, S - lo2
                region = feed_tile(seg_counter[0], c1 + c2)
                seg_counter[0] += 1
                nc.tensor.matmul(region[:, 0:c1], khat[:, lo1:lo1 + 128],
                                 qhat[h][:, lo1:S], start=True, stop=True)
                nc.tensor.matmul(region[:, c1:c1 + c2],
                                 khat[:, lo2:lo2 + 128],
                                 qhat[h][:, lo2:S], start=True, stop=True)
                nc.scalar.activation(
                    et[:, 0:c1 + c2], region[:, 0:c1 + c2],
                    mybir.ActivationFunctionType.Exp, scale=SCALE)
                L(f"mask{h}_{jt}")
                if h == 3:
                    nc.gpsimd.tensor_mul(et[:, 0:128], et[:, 0:128], tri_sb)
                    nc.gpsimd.tensor_mul(et[:, c1:c1 + 128],
                                         et[:, c1:c1 + 128], tri_sb)
                else:
                    nc.vector.tensor_mul(et[:, 0:128], et[:, 0:128], tri_sb)
                    nc.vector.tensor_mul(et[:, c1:c1 + 128],
                                         et[:, c1:c1 + 128], tri_sb)
                return c1

            attnv_state = {}   # h -> bank_first dict

            def attnv_row(h, jt, et, desc=False, ebase=0):
                # In ascending key order slice jt is complete after this row
                # (stop + normalize); in descending order every slice stays
                # open until row 0 and normalization happens afterwards.
                # ebase shifts the et column origin for flat-packed rows.
                L(f"attnv{h}_{jt}")
                bank_first = attnv_state.setdefault(h, {})
                b1_hi = min(jt + 7, NST - 1)
                order = list(range(b1_hi, jt - 1, -1)) + \
                    list(range(NST - 1, b1_hi, -1))
                for it in order:
                    b = it // 7
                    first = b not in bank_first
                    c0 = it * 128 - ebase
                    mm = nc.tensor.matmul(
                        av_slice(it), et[:, c0:c0 + 128],
                        v_sb[:, jt, :],
                        start=first,
                        stop=(jt == 0 if desc else it == jt),
                        skip_group_check=True)
                    if first:
                        bank_first[b] = mm
                    elif jt == 0:
                        add_dep_helper(mm.ins, bank_first[b].ins,
                                       sync=False,
                                       reason="bank clear first")
                if not desc:
                    normalize_slice(h, jt)

            def normalize_slice(h, jt, eng=0, rc=None):
                L(f"norm{h}_{jt}")
                pso = av_slice(jt)
                if rc is None:
                    rc = rcp.tile([128, 1], f32, tag="rc", name="rc")
                    nc.vector.reciprocal(rc, pso[:, HD:HD + 1])
                if eng == 0:
                    nc.vector.tensor_scalar_mul(
                        attn_n[:, jt, h * HD:(h + 1) * HD], pso[:, 0:HD], rc)
                else:
                    nc.scalar.mul(
                        attn_n[:, jt, h * HD:(h + 1) * HD], pso[:, 0:HD], rc)

            # oproj -------------------------------------------------------
            # Column groups g=0..2 keep the original 4-hc-batched [512,512]
            # output DMAs (few HWDGE entries).  Group 3 (seq 1536:2048) is
            # split: a 384-wide part (query tiles 12-14, ready one attnV row
            # before the end) drained + DMA'd flat, and a final 128-wide
            # sliver (tile 15) that is DMA'd directly from PSUM as f32 so
            # the kernel tail is one small transfer with no drain wait.
            oproj_pending = [(g, hc) for g in range(3) for hc in range(KC)]
            ot_state = {}

            def oproj_chunk(ps, drain_eng):
                g, hc = oproj_pending.pop(0)
                L(f"oproj{g}_{hc}")
                col = g * 512
                for kc2 in range(2):
                    nc.tensor.matmul(
                        ps, wo_sb[:, kc2, hc * 128:(hc + 1) * 128],
                        aT[:, kc2, col:col + 512],
                        start=(kc2 == 0), stop=(kc2 == 1))
                if hc % 4 == 0:
                    ot_state[g] = otp.tile([128, 4, 512], bf16, tag="otb",
                                           name="otb")
                ot = ot_state[g]
                with tc.high_priority(offset=-64):
                    if drain_eng == 0:
                        nc.vector.tensor_copy(ot[:, hc % 4, :], ps)
                    elif drain_eng == 1:
                        nc.scalar.copy(ot[:, hc % 4, :], ps)
                    else:
                        nc.vector.tensor_copy(ot[:, hc % 4, 0:256],
                                              ps[:, 0:256])
                        nc.scalar.copy(ot[:, hc % 4, 256:512],
                                       ps[:, 256:512])
                if hc % 4 == 3:
                    r0 = (hc // 4) * 512
                    nc.sync.dma_start(
                        oTd[r0:r0 + 512, col:col + 512].rearrange(
                            "(c p) m -> p c m", p=128), ot)

            g3_pending = list(range(KC))
            ot3_state = {}

            def g3_chunk(ps, drain_eng):
                hc = g3_pending.pop(0)
                L(f"g3_{hc}")
                for kc2 in range(2):
                    nc.tensor.matmul(
                        ps[:, 0:384], wo_sb[:, kc2, hc * 128:(hc + 1) * 128],
                        aT[:, kc2, 1536:1920],
                        start=(kc2 == 0), stop=(kc2 == 1))
                if hc % 4 == 0:
                    ot3_state[hc // 4] = otp.tile(
                        [128, 4, 384], bf16, tag="ot3", name="ot3")
                ot = ot3_state[hc // 4]
                if drain_eng == 0:
                    nc.vector.tensor_copy(ot[:, hc % 4, :], ps[:, 0:384])
                else:
                    nc.scalar.copy(ot[:, hc % 4, :], ps[:, 0:384])
                if hc % 4 == 3:
                    grp = hc // 4
                    nc.sync.dma_start(
                        oT3d[:, grp * 1536:(grp + 1) * 1536], ot)

            # ---- main pipelined loop ------------------------------------
            # head 0 walks key rows DESCENDING (matched to the reverse xt
            # stream: each xt column block unlocks 4 more rows and their
            # k/v/pair-projection chunks); heads 1-3 ascend as before.
            seq = [(0, jt) for jt in range(NST - 1, -1, -1)] + \
                [(h, jt) for h in range(1, G) for jt in range(NST)]
            prev = None
            pair_state = {}
            for (h, jt) in seq:
                if h == 0 and jt in (15, 13):
                    w12 = 2 * S - (2 * jt - 1) * 128
                    et = ep.tile([128, w12], bf16, tag="e",
                                 name=f"e{h}_{jt}")
                    pair_state = {"et": et,
                                  "c1": scores_pair(0, jt, jt - 1, et)}
                    ebase = jt * 128
                elif h == 0 and jt in (14, 12):
                    et = pair_state["et"]
                    ebase = jt * 128 - pair_state["c1"]
                elif h > 0 and jt in (12, 14):
                    # rows (12,13) and (14,15) share one exp instruction
                    w12 = 2 * S - (2 * jt + 1) * 128
                    et = ep.tile([128, w12], bf16, tag="e",
                                 name=f"e{h}_{jt}")
                    pair_state = {"et": et,
                                  "c1": scores_pair(h, jt, jt + 1, et)}
                    ebase = jt * 128
                elif h > 0 and jt in (13, 15):
                    et = pair_state["et"]
                    ebase = jt * 128 - pair_state["c1"]
                else:
                    et = ep.tile([128, S], bf16, tag="e", name=f"e{h}_{jt}")
                    scores_row(h, jt, et)
                    ebase = 0
                if prev is not None:
                    attnv_row(prev[0], prev[1], prev[2], desc=(prev[0] == 0),
                              ebase=prev[3])
                    if prev[:2] == (0, 0):
                        # head 0 ran descending: all 16 attnV slices close
                        # at row 0; normalize split across DVE/ACT, in
                        # descending slice order to pipeline with head 1's
                        # bank-clearing attnV writes (slice 15 first).
                        # Reciprocals batch first so the ACT muls never wait
                        # on an individual DVE recip.
                        rcs = {}
                        for i in range(NST - 1, -1, -1):
                            rcs[i] = rcp.tile([128, 1], f32, tag="rc",
                                              name="rcb")
                            nc.vector.reciprocal(
                                rcs[i], av_slice(i)[:, HD:HD + 1])
                        for i in range(NST - 1, -1, -1):
                            normalize_slice(0, i, eng=0, rc=rcs[i])
                prev = (h, jt, et, ebase)

                # interleaved producer work; head-0 rows chase the reverse
                # xt stream, so the projection bursts sit right after the
                # last row that only needs already-landed data.
                if h == 0:
                    if jt == 15:
                        v_tiles(14, 2, ps=borrow_feed())
                    if jt == 14:
                        v_tiles(12, 2, ps=borrow_feed())
                    if jt == 13:
                        k_proj(2)
                    if jt == 12:
                        k_fold(2)
                        pair_proj(0, 2, ps=borrow_feed())
                        q_permute(0, 2, ps=borrow_feed())
                    if jt == 11:
                        v_tiles(8, 4, ps=borrow_feed())
                    if jt == 9:
                        k_proj(1)
                    if jt == 8:
                        k_fold(1)
                        pair_proj(0, 1, ps=borrow_feed())
                        q_permute(0, 1, ps=borrow_feed())
                    if jt == 7:
                        v_tiles(4, 4)
                    if jt == 5:
                        k_proj(0)
                    if jt == 4:
                        k_fold(0)
                        pair_proj(0, 0, ps=borrow_feed())
                        q_permute(0, 0, ps=borrow_feed())
                    if jt == 3:
                        v_tiles(0, 4)
                    if jt in (3, 2, 1, 0):
                        q_permute(1, 3 - jt)
                if h == 1:
                    if jt in (1, 4, 7, 10):
                        pair_proj(1, (jt - 1) // 3)
                    if jt in (2, 5, 8, 11):
                        q_permute(2, (jt - 2) // 3)
                if h == 2 and jt in (1, 4, 7, 10):
                    q_permute(3, (jt - 1) // 3)
                if h == 2 and jt in (3, 7, 11, 15):
                    transpose_group(0, jt // 4)
                if h == 3:
                    if jt in (5, 9, 13):
                        transpose_group(1, (jt - 5) // 4)
                    if jt >= 5 and oproj_pending and \
                            oproj_pending[0][0] * 4 + 5 <= jt:
                        ps = pw.tile([128, 512], f32, tag="w", name="psow")
                        oproj_chunk(ps, drain_eng=0)
                    if jt >= 8 and oproj_pending and \
                            oproj_pending[0][0] * 4 + 5 <= jt:
                        ps = pav.tile([128, 512], f32, tag="av0",
                                      name="psoa")
                        oproj_chunk(ps, drain_eng=1 if jt >= 12 else 0)
                    if jt in (11, 13, 15) and oproj_pending and \
                            oproj_pending[0][0] * 4 + 5 <= jt:
                        ps = pav.tile([128, 512], f32, tag="av0",
                                      name="psoa2")
                        oproj_chunk(ps, drain_eng=1)
                    if jt in (14, 15) and oproj_pending and \
                            oproj_pending[0][0] * 4 + 5 <= jt:
                        ps = pav.tile([128, 512], f32, tag="av1",
                                      name="psob")
                        oproj_chunk(ps, drain_eng=1)
                    if jt == 15:
                        # rows 12-14 of head 3 are normalized; pair-1
                        # transposes for tiles 12-14 unblock the 384-wide
                        # part of column group 3.
                        transpose_tiles(1, [12, 13, 14])

            # flush: last attnV row, then the 384-wide part of group 3
            # (query tiles 12-14), the tile-15 transpose, and the sliver.
            attnv_row(prev[0], prev[1], prev[2], ebase=prev[3])
            ti = 0
            slots = ["fA", "fB", "av0", "w", "av1"]
            pools = {"w": pw, "fA": pfa, "fB": pfb, "av0": pav, "av1": pav}
            while oproj_pending:
                tag = slots[ti % len(slots)]
                ps = pools[tag].tile([128, 512], f32, tag=tag, name="psot")
                oproj_chunk(ps, drain_eng=ti % 2)
                ti += 1

            def g3_next(drain_eng):
                tag = slots[ti % len(slots)]
                ps = pools[tag].tile([128, 512], f32, tag=tag, name="psog3")
                g3_chunk(ps, drain_eng)

            # two g3 chunks cover the normalize latency of row 15, then the
            # tile-15 transpose slots in, then the rest.
            g3_next(0)
            g3_next(1)
            ti += 2
            transpose_tiles(1, [15])
            while g3_pending:
                g3_next(ti % 2)
                ti += 1
            del g3_next

            # sliver: 8 feature chunks x 128 seq cols; two [128, 512] f32
            # psum tiles, drained on parallel engines, two small flat DMAs.
            for half in range(2):
                L(f"sliver{half}")
                psl = (pfa if half == 0 else pfb).tile(
                    [128, 512], f32, tag=("fA" if half == 0 else "fB"),
                    name="psliv", padded_shape=[128, 1024])
                for sub in range(4):
                    hc = half * 4 + sub
                    for kc2 in range(2):
                        nc.tensor.matmul(
                            psl[:, sub * 128:(sub + 1) * 128],
                            wo_sb[:, kc2, hc * 128:(hc + 1) * 128],
                            aT[:, kc2, 1920:2048],
                            start=(kc2 == 0), stop=(kc2 == 1))
                ot2 = otp.tile([128, 512], bf16, tag="ot2", name="ot2")
                if half == 0:
                    nc.vector.tensor_copy(ot2, psl)
                else:
                    nc.scalar.copy(ot2, psl)
                nc.sync.dma_start(
                    oT2d[:, half * 512:(half + 1) * 512], ot2)

    nc.finalize()
    return nc


def _host_inputs(hidden_states, position_ids, wq, wk, wv, wo):
    """Build the 8 per-core input maps."""
    def w2_of(w):
        # w: [64, H] rows of one head; returns sign-permuted rows
        w2 = np.empty_like(w)
        w2[:32] = -w[32:64]
        w2[32:] = w[:32]
        return w2

    trimask = np.triu(np.ones((128, 128), np.float32)).astype(BF16)
    dupJ = np.zeros((128, 128), np.float32)
    for p in range(128):
        dupJ[p, p % 64] = 1.0
        dupJ[p, p % 64 + 64] = 1.0
    dupJ = dupJ.astype(BF16)
    ident = np.eye(128, dtype=np.float32).astype(BF16)

    # perm[lh][k, p]: rows of the packed pair projection (head lh occupies
    # input rows 64*lh..64*lh+63) -> [q; sign-permuted q] output rows
    perms = []
    for lh in range(2):
        P = np.zeros((128, 128), np.float32)
        b = 64 * lh
        for p in range(64):
            P[b + p, p] = 1.0
        for i in range(32):
            P[b + 32 + i, 64 + i] = -1.0
            P[b + i, 96 + i] = 1.0
        perms.append(P.astype(BF16))

    def swz(wT):
        # [H, 128] -> [128, KC*128] with 2KB-contiguous DMA descriptors
        return np.ascontiguousarray(
            wT.reshape(KC, 128, 128).transpose(1, 0, 2)
            .reshape(128, KC * 128)).astype(BF16)

    in_maps = []
    for core in range(N_CORES):
        b, kv = core // NKV, core % NKV
        xT = np.ascontiguousarray(hidden_states[b].T).astype(BF16)

        wkh = wk[kv * HD:(kv + 1) * HD]
        wkT8 = swz(np.concatenate([wkh.T, w2_of(wkh).T], axis=1))
        wpT8 = []
        for pr in range(2):
            h0 = kv * G + 2 * pr
            wpT8.append(swz(wq[h0 * HD:(h0 + 2) * HD].T))

        wvT = np.ascontiguousarray(wv[kv * HD:(kv + 1) * HD].T).astype(BF16)
        woT = np.ascontiguousarray(
            wo[:, kv * G * HD:(kv + 1) * G * HD].T).astype(BF16)

        inv = 1.0 / (THETA ** (np.arange(0, HD, 2, dtype=np.float32) / HD))
        freqs = position_ids[b].astype(np.float32)[:, None] * inv[None, :]
        emb = np.concatenate([freqs, freqs], axis=-1)       # [S, 64]
        cs = np.concatenate([np.cos(emb).T, np.sin(emb).T], axis=0)  # [128, S]
        cs = np.ascontiguousarray(cs).astype(BF16)

        in_maps.append({
            "xT": xT, "wvT": wvT, "cs": cs, "woT": woT,
            "trimask": trimask, "dupJ": dupJ, "ident": ident,
            "wkT8": wkT8, "wp0T8": wpT8[0], "wp1T8": wpT8[1],
            "perm0": perms[0], "perm1": perms[1],
        })
    return in_maps


_NC_CACHE = {}


def run_cores(in_maps, trace=False, trace_kwargs=None):
    from concourse.bass_utils import run_bass_kernel_spmd
    if "nc" not in _NC_CACHE:
        _NC_CACHE["nc"] = _build_nc()
    nc = _NC_CACHE["nc"]
    return run_bass_kernel_spmd(
        nc, in_maps, core_ids=list(range(N_CORES)),
        trace=trace, **(trace_kwargs or {}))


def assemble(res):
    """Gather the 3 per-core output pieces into the full [B, S, H] output."""
    out = np.zeros((B, S, H), np.float32)
    for core in range(N_CORES):
        b = core // NKV
        r = res.results[core]
        out[b, 0:1536] += r["oT"].T.astype(np.float32)
        out[b, 1536:1920] += (
            r["oT3"].reshape(128, 2, 4, 384).transpose(3, 1, 2, 0)
            .reshape(384, H).astype(np.float32))
        out[b, 1920:2048] += (
            r["oT2"].reshape(128, 8, 128).transpose(2, 1, 0)
            .reshape(128, H).astype(np.float32))
    return out


def kernel(hidden_states, attention_mask, position_ids, wq, wk, wv, wo):
    hidden_states = np.asarray(hidden_states, dtype=np.float32)
    position_ids = np.asarray(position_ids)
    wq = np.asarray(wq, dtype=np.float32)
    wk = np.asarray(wk, dtype=np.float32)
    wv = np.asarray(wv, dtype=np.float32)
    wo = np.asarray(wo, dtype=np.float32)

    in_maps = _host_inputs(hidden_states, position_ids, wq, wk, wv, wo)
    res = run_cores(in_maps)
    return assemble(res)

